# revision 3
# baseline (speedup 1.0000x reference)
"""Trainium2 Bass kernel for nn_HandIntersectionLoss.

Strategy
--------
Pure data parallel over batch: 64 batches -> 8 cores x 8 local batches.

The reference math is reformulated so the tensor engine does the heavy
per-(point, face) lifting via K=5 matmuls (polynomial expansion of the
Van Oosterom / Strackee solid-angle terms):

    |A-p|^2          = |A|^2 - 2 p.A + |p|^2
    (A-p).(B-p)      = A.B - p.(A+B) + |p|^2
    det(A-p,B-p,C-p) = A.(BxC) - p.(AxB + BxC + CxA)

With moving rows [-2px,-2py,-2pz, 1, |p|^2] a single matmul against
per-face constant columns produces la^2, lb^2, lc^2, ab, bc, ca, det
for a [128 points x 500 faces] block.  The per-element chain
(denominator assembly + range-reduced atan2) runs on DVE/ACT:

    atan2(det, den) = 2*atan(det / (rho + |den|))            (den >= 0)
                    = sign(det)*pi - 2*atan(det/(rho+|den|)) (den < 0)
    rho = sqrt(det^2 + den^2 + 1e-20)   -> |atan input| <= 1 always

inside(p) <=> sum_f atan2 > pi <=> sum_f half > pi/2.  Min-distance
uses the same matmul trick + free-dim min-reduce.

Scalar-engine table sets force a two-pass structure (sqrt and arctan
live in different ACT table sets): pass A computes through tt=det/dd
(sqrt set), pass B does the arctan + quadrant correction (sigmoid set),
with den/tt staged in SBUF between passes (super-groups of 16 blocks to
fit the SBUF column budget).

Host side does only index gathers / constant prep (O(B*F)) - all
O(B*P*F) math runs on device.
"""
import os
import sys
import numpy as np

sys.path.insert(0, '/opt/trn_rl_repo')

B, V_FULL, V_HAND, V_LOOP, N_FACES = 64, 6890, 250, 20, 500
P = V_HAND + 1          # 251 points/verts per hand (incl. lid)
PPAD = 256
NCORES = 8
NB = B // NCORES        # local batches per core
NBD = NB * 2            # (batch, dir) pairs per core
NBLK = NBD * 2          # blocks per core: x2 point-chunks of 128
SUPER = 16              # blocks per two-pass super-group
F = N_FACES
HALF_PI = float(np.pi / 2)

_compiled = None        # cached compiled program across kernel() calls
last_exec_time_ns = None


# --------------------------------------------------------------------------
# host prep: index gathers + per-face constants (float64 -> float32 round)
# --------------------------------------------------------------------------

def _host_prep(inputs):
    verts = np.asarray(inputs['verts_batch'], dtype=np.float32)
    idx = {k: np.asarray(inputs[k], dtype=np.int64) for k in (
        'hand_verts_inds_left', 'hand_verts_inds_right',
        'hand_loop_verts_inds_left', 'hand_loop_verts_inds_right',
        'hand_faces_left', 'hand_faces_right')}

    pts = {}
    for d, (hi, li) in enumerate([
            ('hand_verts_inds_left', 'hand_loop_verts_inds_left'),
            ('hand_verts_inds_right', 'hand_loop_verts_inds_right')]):
        h = verts[:, idx[hi]]                                   # [B,250,3]
        lid = verts[:, idx[li]].mean(axis=1, keepdims=True, dtype=np.float32)
        pts[d] = np.concatenate([h, lid], axis=1)               # [B,251,3] f32

    faces = {0: idx['hand_faces_left'], 1: idx['hand_faces_right']}

    lhsT = np.zeros((B, 2, 5, PPAD), np.float32)
    frhs = np.zeros((B, 2, 5, 7, 512), np.float32)   # [.., K-row, group, face]
    mrhs = np.zeros((B, 2, 5, PPAD), np.float32)

    for d in range(2):
        p = pts[d].astype(np.float64)
        pad = np.full((B, PPAD - P, 3), 1e3)
        pf = np.concatenate([p, pad], axis=1)                   # [B,256,3]
        lhsT[:, d, 0:3] = (-2.0 * pf.transpose(0, 2, 1)).astype(np.float32)
        lhsT[:, d, 3] = 1.0
        lhsT[:, d, 4] = (pf ** 2).sum(-1).astype(np.float32)

        ov = pts[1 - d].astype(np.float64)                      # other-hand verts
        tri = ov[:, faces[1 - d]]                               # [B,500,3,3]
        A, Bv, C = tri[:, :, 0], tri[:, :, 1], tri[:, :, 2]
        n = np.cross(A, Bv) + np.cross(Bv, C) + np.cross(C, A)
        d0 = np.einsum('bfi,bfi->bf', A, np.cross(Bv, C))
        groups = [
            (A,            (A ** 2).sum(-1),                1.0),
            (Bv,           (Bv ** 2).sum(-1),               1.0),
            (C,            (C ** 2).sum(-1),                1.0),
            ((A + Bv) / 2, np.einsum('bfi,bfi->bf', A, Bv), 1.0),
            ((Bv + C) / 2, np.einsum('bfi,bfi->bf', Bv, C), 1.0),
            ((C + A) / 2,  np.einsum('bfi,bfi->bf', C, A),  1.0),
            (n / 2,        d0,                              0.0),
        ]
        for g, (xyz, c3, ones) in enumerate(groups):
            frhs[:, d, 0:3, g, :F] = xyz.transpose(0, 2, 1).astype(np.float32)
            frhs[:, d, 3, g, :F] = c3.astype(np.float32)
            frhs[:, d, 4, g, :F] = ones

        mrhs[:, d, 0:3, :P] = ov.transpose(0, 2, 1).astype(np.float32)
        mrhs[:, d, 3, :P] = (ov ** 2).sum(-1).astype(np.float32)
        mrhs[:, d, 4, :P] = 1.0

    return lhsT, frhs, mrhs


# --------------------------------------------------------------------------
# device kernel
# --------------------------------------------------------------------------

def _kernel_body(tc, lhsT_d, frhs_d, mrhs_d, loss_d, dbg=None):
    import concourse.mybir as mybir
    nc = tc.nc
    fp32 = mybir.dt.float32
    AF = mybir.ActivationFunctionType
    OP = mybir.AluOpType
    AX = mybir.AxisListType.X

    with (
        tc.tile_pool(name="const", bufs=1) as cpool,
        tc.tile_pool(name="store", bufs=1) as spool,
        tc.tile_pool(name="stage", bufs=2) as stpool,
        tc.tile_pool(name="iface", bufs=2) as ipool,
        tc.tile_pool(name="dve", bufs=1) as vpool,
    ):
        lhsT_sb = cpool.tile([5, NBD, PPAD], fp32)
        nc.sync.dma_start(lhsT_sb[:], lhsT_d[:])

        ones = cpool.tile([128, 1], fp32)
        nc.vector.memset(ones[:], 1.0)

        sacc = cpool.tile([128, NBLK], fp32)     # per block: sum_f half-angle
        minda = cpool.tile([128, NBLK], fp32)    # per block: clamped min d^2
        denoms = spool.tile([128, SUPER, 512], fp32)
        tts = spool.tile([128, SUPER, 512], fp32)

        def pass_a(ppool, i, j):
            bd, ch = divmod(i, 2)
            if ch == 0:
                fstage = stpool.tile([5, 7, 512], fp32, tag="fstage")
                mstage = stpool.tile([5, PPAD], fp32, tag="mstage")
                nc.sync.dma_start(fstage[:], frhs_d[:, bd])
                nc.sync.dma_start(mstage[:], mrhs_d[:, bd])
                pass_a.stage = (fstage, mstage)
            fstage, mstage = pass_a.stage
            lhs = lhsT_sb[:, bd, ch * 128:(ch + 1) * 128]       # [5,128]

            wind = ppool.tile([128, 7, 512], fp32, tag="wind")
            md = ppool.tile([128, 256], fp32, tag="md")

            for g in range(7):
                nc.tensor.matmul(wind[:, g, :F], lhs, fstage[:, g, :F])
            nc.tensor.matmul(md[:, :P], lhs, mstage[:, :P])

            # min-distance: free-dim min, clamp at 0 (matmul roundoff)
            mind = vpool.tile([128, 1], fp32, tag="mind")
            nc.vector.tensor_reduce(mind[:], md[:, :P], AX, OP.min)
            nc.vector.tensor_scalar(minda[:, i:i + 1], mind[:], 0.0, None,
                                    OP.max)

            # norms: clamp squared lengths at 0 (fp32 matmul roundoff), sqrt
            rl = ipool.tile([128, 3, 512], fp32, tag="rl")
            for g in range(3):
                nc.scalar.activation(rl[:, g, :F], wind[:, g, :F], AF.Relu)
            la = ipool.tile([128, 512], fp32, tag="la")
            lb = ipool.tile([128, 512], fp32, tag="lb")
            lc = ipool.tile([128, 512], fp32, tag="lc")
            nc.scalar.activation(la[:, :F], rl[:, 0, :F], AF.Sqrt)
            nc.scalar.activation(lb[:, :F], rl[:, 1, :F], AF.Sqrt)
            nc.scalar.activation(lc[:, :F], rl[:, 2, :F], AF.Sqrt)
            dets = ipool.tile([128, 512], fp32, tag="dets")
            nc.scalar.activation(dets[:, :F], wind[:, 6, :F], AF.Copy)

            # denominator chain (DVE); PSUM reads scheduled early
            u = vpool.tile([128, 512], fp32, tag="u")
            r4 = vpool.tile([128, 512], fp32, tag="r4")
            s5 = vpool.tile([128, 512], fp32, tag="s5")
            v = vpool.tile([128, 512], fp32, tag="v")
            w = vpool.tile([128, 512], fp32, tag="w")
            t6 = vpool.tile([128, 512], fp32, tag="t6")
            nc.vector.tensor_tensor(r4[:, :F], wind[:, 4, :F], la[:, :F],
                                    OP.mult)
            nc.vector.tensor_tensor(s5[:, :F], wind[:, 5, :F], lb[:, :F],
                                    OP.mult)
            nc.vector.tensor_tensor(u[:, :F], la[:, :F], lb[:, :F], OP.mult)
            nc.vector.tensor_tensor(v[:, :F], u[:, :F], wind[:, 3, :F],
                                    OP.add)

            # rest of the chain is SBUF-only
            w_ = w[:, :F]
            nc.vector.tensor_tensor(w_, v[:, :F], lc[:, :F], OP.mult)
            nc.vector.tensor_tensor(t6[:, :F], r4[:, :F], s5[:, :F], OP.add)
            den = denoms[:, j, :F]
            nc.vector.tensor_tensor(den, w_, t6[:, :F], OP.add)

            # half-angle atan2 range reduction: tt = det / (rho + |den|)
            xx = ipool.tile([128, 512], fp32, tag="xx")
            yy = ipool.tile([128, 512], fp32, tag="yy")
            ss = vpool.tile([128, 512], fp32, tag="ss", bufs=2)
            rho = ipool.tile([128, 512], fp32, tag="rho")
            axd = ipool.tile([128, 512], fp32, tag="axd")
            dd = vpool.tile([128, 512], fp32, tag="dd")
            rd = vpool.tile([128, 512], fp32, tag="rd")
            nc.scalar.activation(xx[:, :F], den, AF.Square)
            nc.scalar.activation(yy[:, :F], dets[:, :F], AF.Square)
            nc.vector.scalar_tensor_tensor(ss[:, :F], xx[:, :F], 1e-20,
                                           yy[:, :F], OP.add, OP.add)
            nc.scalar.activation(rho[:, :F], ss[:, :F], AF.Sqrt)
            nc.scalar.activation(axd[:, :F], den, AF.Abs)
            nc.vector.tensor_tensor(dd[:, :F], rho[:, :F], axd[:, :F], OP.add)
            nc.vector.reciprocal_approx_fast(rd[:, :F], dd[:, :F])
            nc.vector.tensor_tensor(tts[:, j, :F], dets[:, :F], rd[:, :F],
                                    OP.mult)
            if dbg is not None and i == 0:
                wcopy = vpool.tile([128, 7, 512], fp32, tag="wcopy")
                for g in range(7):
                    nc.scalar.activation(wcopy[:, g, :F], wind[:, g, :F], AF.Copy)
                    nc.sync.dma_start(dbg["wind0"][:, g, :F], wcopy[:, g, :F])
                nc.sync.dma_start(dbg["den0"][:, :F], denoms[:, 0, :F])
                nc.sync.dma_start(dbg["tt0"][:, :F], tts[:, 0, :F])

        def pass_b(i, j):
            den = denoms[:, j, :F]
            tt = tts[:, j, :F]
            sgn = ipool.tile([128, 512], fp32, tag="sgn")
            spi = ipool.tile([128, 512], fp32, tag="spi")
            atn = ipool.tile([128, 512], fp32, tag="atn")
            c0 = vpool.tile([128, 512], fp32, tag="c0")
            c1 = vpool.tile([128, 512], fp32, tag="c1")
            sd = vpool.tile([128, 512], fp32, tag="sd")
            nc.scalar.activation(sgn[:, :F], tt, AF.Sign)
            nc.scalar.mul(spi[:, :F], sgn[:, :F], HALF_PI)
            nc.scalar.activation(atn[:, :F], tt, AF.Arctan)
            # half = atn + [den<0]*(pi/2*sign(det) - 2*atn); sign(det)==sign(atn)
            # (gpsimd offload of these was tried: fails in the bass2jax/PJRT
            # lowering, so they stay on DVE)
            nc.vector.scalar_tensor_tensor(c0[:, :F], atn[:, :F], -2.0,
                                           spi[:, :F], OP.mult, OP.add)
            nc.vector.scalar_tensor_tensor(c1[:, :F], den, 0.0,
                                           c0[:, :F], OP.is_lt, OP.mult)
            nc.vector.scalar_tensor_tensor(sd[:, :F], atn[:, :F], 0.0,
                                           c1[:, :F], OP.add, OP.add,
                                           accum_out=sacc[:, i:i + 1])

        with tc.tile_pool(name="psum", bufs=1, space="PSUM") as ppool:
            for s in range(NBLK // SUPER):
                for j in range(SUPER):
                    pass_a(ppool, s * SUPER + j, j)
                tc.no_sync_barrier()
                for j in range(SUPER):
                    pass_b(s * SUPER + j, j)
                tc.no_sync_barrier()

        # ---------------- final: depth * inside, partition-reduce ----------
        inside = cpool.tile([128, NBLK], fp32)
        depth = cpool.tile([128, NBLK], fp32)
        contrib = cpool.tile([128, NBLK], fp32)
        beps = cpool.tile([128, 1], fp32)
        nc.vector.memset(beps[:], 1e-12)
        nc.vector.tensor_scalar(inside[:], sacc[:], HALF_PI, None, OP.is_gt)
        nc.scalar.activation(depth[:], minda[:], AF.Sqrt, bias=beps[:])
        nc.vector.tensor_tensor(contrib[:], depth[:], inside[:], OP.mult)

        with tc.tile_pool(name="psum2", bufs=1, space="PSUM") as p2:
            lpsum = p2.tile([NBLK, 1], fp32)
            nc.tensor.matmul(lpsum[:], contrib[:], ones[:])
            loss_sb = cpool.tile([NBLK, 1], fp32)
            nc.scalar.activation(loss_sb[:], lpsum[:], AF.Copy)
            nc.sync.dma_start(loss_d[:], loss_sb[:])
        if dbg is not None:
            nc.sync.dma_start(dbg["sacc"][:], sacc[:])
            nc.sync.dma_start(dbg["minda"][:], minda[:])


def _build():
    global _compiled
    if _compiled is not None:
        return _compiled
    import concourse.bacc as bacc
    import concourse.mybir as mybir
    import concourse.tile as tile

    nc = bacc.Bacc("TRN2", target_bir_lowering=False, debug=False,
                   num_devices=NCORES)
    fp32 = mybir.dt.float32
    lhsT_d = nc.dram_tensor("lhsT", (5, NBD, PPAD), fp32, kind="ExternalInput").ap()
    frhs_d = nc.dram_tensor("frhs", (5, NBD, 7, 512), fp32, kind="ExternalInput").ap()
    mrhs_d = nc.dram_tensor("mrhs", (5, NBD, PPAD), fp32, kind="ExternalInput").ap()
    loss_d = nc.dram_tensor("loss", (NBLK, 1), fp32, kind="ExternalOutput").ap()

    with tile.TileContext(nc) as tc:
        _kernel_body(tc, lhsT_d, frhs_d, mrhs_d, loss_d)
    nc.compile()
    _compiled = nc
    return nc


# --------------------------------------------------------------------------
# dispatch: jit(shard_map(bass_exec)) built ONCE and cached.
#
# The stock run_bass_kernel_spmd -> run_bass_via_pjrt path creates a fresh
# jax.jit closure on every call, so each kernel() invocation pays a full
# retrace + relower + executable lookup.  Building the jitted callable once
# and reusing it drops per-call overhead to input transfer + execute.
# --------------------------------------------------------------------------

_dispatch = None


def _get_dispatch():
    global _dispatch
    if _dispatch is not None:
        return _dispatch
    import jax
    from jax.experimental.shard_map import shard_map
    from jax.sharding import Mesh, PartitionSpec
    from concourse import bass2jax
    import concourse.mybir as mybir

    nc = _build()
    bass2jax.install_neuronx_cc_hook()
    assert nc.partition_id_tensor is None and nc.dbg_addr is None

    in_names, out_names, out_avals = [], [], []
    for alloc in nc.m.functions[0].allocations:
        if not isinstance(alloc, mybir.MemoryLocationSet):
            continue
        name = alloc.memorylocations[0].name
        if alloc.kind == "ExternalInput":
            in_names.append(name)
        elif alloc.kind == "ExternalOutput":
            out_names.append(name)
            out_avals.append(jax.core.ShapedArray(
                tuple(alloc.tensor_shape), mybir.dt.np(alloc.dtype)))
    n_params = len(in_names)
    all_names = tuple(in_names) + tuple(out_names)

    def _body(*args):
        outs = bass2jax._bass_exec_p.bind(
            *args,
            out_avals=tuple(out_avals),
            in_names=all_names,
            out_names=tuple(out_names),
            lowering_input_output_aliases=(),
            sim_require_finite=True,
            sim_require_nnan=True,
            nc=nc,
        )
        return tuple(outs)

    devices = jax.devices()[:NCORES]
    mesh = Mesh(np.asarray(devices), ("core",))
    in_specs = (PartitionSpec("core"),) * (n_params + len(out_names))
    out_specs = (PartitionSpec("core"),) * len(out_names)
    donate = tuple(range(n_params, n_params + len(out_names)))
    sharded = jax.jit(
        shard_map(_body, mesh=mesh, in_specs=in_specs,
                  out_specs=out_specs, check_rep=False),
        donate_argnums=donate, keep_unused=True)
    shapes = {"lhsT": (NCORES * 5, NBD, PPAD),
              "frhs": (NCORES * 5, NBD, 7, 512),
              "mrhs": (NCORES * 5, NBD, PPAD)}
    _dispatch = (sharded, tuple(in_names), shapes)
    return _dispatch


def _global_feed(lhsT, frhs, mrhs):
    # per-core [5, NBD, ...] stacked on axis 0 -> [NCORES*5, NBD, ...]
    def to_global(a, tail):
        return np.ascontiguousarray(
            a.reshape((NCORES, NBD, 5) + tail).transpose(0, 2, 1, 3)
             .reshape((NCORES * 5, NBD) + tail))
    return {
        "lhsT": to_global(lhsT.reshape(B, 2, 5, PPAD), (PPAD,)),
        "frhs": np.ascontiguousarray(
            frhs.reshape(NCORES, NBD, 5, 7 * 512).transpose(0, 2, 1, 3)
                .reshape(NCORES * 5, NBD, 7, 512)),
        "mrhs": to_global(mrhs.reshape(B, 2, 5, PPAD), (PPAD,)),
    }


def kernel(**inputs) -> np.ndarray:
    global last_exec_time_ns
    lhsT, frhs, mrhs = _host_prep(inputs)

    if bool(int(os.environ.get("HAND_KERNEL_TRACE", "0"))):
        # profiling path: stock spmd runner with NTFF tracing
        from concourse.bass_utils import run_bass_kernel_spmd
        nc = _build()
        maps = []
        for c in range(NCORES):
            bs = slice(c * NB, (c + 1) * NB)
            maps.append({
                "lhsT": lhsT[bs].reshape(NBD, 5, PPAD).transpose(1, 0, 2).copy(),
                "frhs": frhs[bs].reshape(NBD, 5, 7, 512).transpose(1, 0, 2, 3).copy(),
                "mrhs": mrhs[bs].reshape(NBD, 5, PPAD).transpose(1, 0, 2).copy(),
            })
        res = run_bass_kernel_spmd(nc, maps, list(range(NCORES)), trace=True)
        last_exec_time_ns = res.exec_time_ns
        loss = np.zeros(B, np.float32)
        for c in range(NCORES):
            out = np.asarray(res.results[c]["loss"], np.float32).reshape(NBLK)
            loss[c * NB:(c + 1) * NB] = out.reshape(NB, 4).sum(axis=1)
        return loss

    sharded, in_names, _shapes = _get_dispatch()
    feed = _global_feed(lhsT, frhs, mrhs)
    (out,) = sharded(*[feed[n] for n in in_names],
                     np.zeros((NCORES * NBLK, 1), np.float32))
    last_exec_time_ns = None
    # block i = (b_loc*2 + dir)*2 + chunk
    return np.asarray(out, np.float32).reshape(B, 4).sum(axis=1)



# revision 5
# speedup vs baseline: 1.6304x; 1.6304x over previous
"""Trainium2 Bass kernel for nn_HandIntersectionLoss.

Strategy
--------
Pure data parallel over batch: 64 batches -> 8 cores x 8 local batches.

The reference math is reformulated so the tensor engine does the heavy
per-(point, face) lifting via K=5 matmuls (polynomial expansion of the
Van Oosterom / Strackee solid-angle terms):

    |A-p|^2          = |A|^2 - 2 p.A + |p|^2
    (A-p).(B-p)      = A.B - p.(A+B) + |p|^2
    det(A-p,B-p,C-p) = A.(BxC) - p.(AxB + BxC + CxA)

With moving rows [-2px,-2py,-2pz, 1, |p|^2] a single matmul against
per-face constant columns produces la^2, lb^2, lc^2, ab, bc, ca, det
for a [128 points x 500 faces] block.  The per-element chain
(denominator assembly + range-reduced atan2) runs on DVE/ACT:

    atan2(det, den) = 2*atan(det / (rho + |den|))            (den >= 0)
                    = sign(det)*pi - 2*atan(det/(rho+|den|)) (den < 0)
    rho = sqrt(det^2 + den^2 + 1e-20)   -> |atan input| <= 1 always

inside(p) <=> sum_f atan2 > pi <=> sum_f half > pi/2.  Min-distance
uses the same matmul trick + free-dim min-reduce.

Scalar-engine table sets force a two-pass structure (sqrt and arctan
live in different ACT table sets): pass A computes through tt=det/dd
(sqrt set), pass B does the arctan + quadrant correction (sigmoid set),
with den/tt staged in SBUF between passes (super-groups of 16 blocks to
fit the SBUF column budget).

Host side does only index gathers / constant prep (O(B*F)) - all
O(B*P*F) math runs on device.
"""
import os
import sys
import numpy as np

sys.path.insert(0, '/opt/trn_rl_repo')

B, V_FULL, V_HAND, V_LOOP, N_FACES = 64, 6890, 250, 20, 500
P = V_HAND + 1          # 251 points/verts per hand (incl. lid)
PPAD = 256
NCORES = 8
NB = B // NCORES        # local batches per core
NBD = NB * 2            # (batch, dir) pairs per core
NBLK = NBD * 2          # blocks per core: x2 point-chunks of 128
SUPER = 16              # blocks per two-pass super-group
F = N_FACES
HALF_PI = float(np.pi / 2)

_compiled = None        # cached compiled program across kernel() calls
last_exec_time_ns = None


# --------------------------------------------------------------------------
# host prep: index gathers + per-face constants (float64 -> float32 round)
# --------------------------------------------------------------------------

def _host_prep(inputs):
    verts = np.asarray(inputs['verts_batch'], dtype=np.float32)
    idx = {k: np.asarray(inputs[k], dtype=np.int64) for k in (
        'hand_verts_inds_left', 'hand_verts_inds_right',
        'hand_loop_verts_inds_left', 'hand_loop_verts_inds_right',
        'hand_faces_left', 'hand_faces_right')}

    pts = {}
    for d, (hi, li) in enumerate([
            ('hand_verts_inds_left', 'hand_loop_verts_inds_left'),
            ('hand_verts_inds_right', 'hand_loop_verts_inds_right')]):
        h = verts[:, idx[hi]]                                   # [B,250,3]
        lid = verts[:, idx[li]].mean(axis=1, keepdims=True, dtype=np.float32)
        pts[d] = np.concatenate([h, lid], axis=1)               # [B,251,3] f32

    faces = {0: idx['hand_faces_left'], 1: idx['hand_faces_right']}

    lhsT = np.zeros((B, 2, 5, PPAD), np.float32)
    frhs = np.zeros((B, 2, 5, 7, 512), np.float32)   # [.., K-row, group, face]
    mrhs = np.zeros((B, 2, 5, PPAD), np.float32)

    for d in range(2):
        p = pts[d].astype(np.float64)
        pad = np.full((B, PPAD - P, 3), 1e3)
        pf = np.concatenate([p, pad], axis=1)                   # [B,256,3]
        lhsT[:, d, 0:3] = (-2.0 * pf.transpose(0, 2, 1)).astype(np.float32)
        lhsT[:, d, 3] = 1.0
        lhsT[:, d, 4] = (pf ** 2).sum(-1).astype(np.float32)

        ov = pts[1 - d].astype(np.float64)                      # other-hand verts
        tri = ov[:, faces[1 - d]]                               # [B,500,3,3]
        A, Bv, C = tri[:, :, 0], tri[:, :, 1], tri[:, :, 2]
        n = np.cross(A, Bv) + np.cross(Bv, C) + np.cross(C, A)
        d0 = np.einsum('bfi,bfi->bf', A, np.cross(Bv, C))
        groups = [
            (A,            (A ** 2).sum(-1),                1.0),
            (Bv,           (Bv ** 2).sum(-1),               1.0),
            (C,            (C ** 2).sum(-1),                1.0),
            ((A + Bv) / 2, np.einsum('bfi,bfi->bf', A, Bv), 1.0),
            ((Bv + C) / 2, np.einsum('bfi,bfi->bf', Bv, C), 1.0),
            ((C + A) / 2,  np.einsum('bfi,bfi->bf', C, A),  1.0),
            (n / 2,        d0,                              0.0),
        ]
        for g, (xyz, c3, ones) in enumerate(groups):
            frhs[:, d, 0:3, g, :F] = xyz.transpose(0, 2, 1).astype(np.float32)
            frhs[:, d, 3, g, :F] = c3.astype(np.float32)
            frhs[:, d, 4, g, :F] = ones

        mrhs[:, d, 0:3, :P] = ov.transpose(0, 2, 1).astype(np.float32)
        mrhs[:, d, 3, :P] = (ov ** 2).sum(-1).astype(np.float32)
        mrhs[:, d, 4, :P] = 1.0

    return lhsT, frhs, mrhs


def _host_prep_global(inputs):
    """float32 host prep writing the sharded global layout directly.

    Returns {"lhsT": [NCORES*5, NBD, PPAD], "frhs": [NCORES*5, NBD, 7, 512],
    "mrhs": [NCORES*5, NBD, PPAD]} where axis 0 stacks (core, K-row) and
    NBD = (local batch, dir) flattened as nb*2 + d.
    """
    verts = np.asarray(inputs['verts_batch'], dtype=np.float32)

    pts = {}
    for d, (hi, li) in enumerate([
            ('hand_verts_inds_left', 'hand_loop_verts_inds_left'),
            ('hand_verts_inds_right', 'hand_loop_verts_inds_right')]):
        h = verts[:, np.asarray(inputs[hi])]                    # [B,250,3]
        lid = verts[:, np.asarray(inputs[li])].mean(
            axis=1, keepdims=True, dtype=np.float32)
        pts[d] = np.concatenate([h, lid], axis=1)               # [B,251,3]

    faces = {0: np.asarray(inputs['hand_faces_left']),
             1: np.asarray(inputs['hand_faces_right'])}

    glhsT = np.zeros((NCORES, 5, NB, 2, PPAD), np.float32)
    gfrhs = np.zeros((NCORES, 5, NB, 2, 7, 512), np.float32)
    gmrhs = np.zeros((NCORES, 5, NB, 2, PPAD), np.float32)

    for d in range(2):
        pf = np.full((B, PPAD, 3), 1e3, np.float32)
        pf[:, :P] = pts[d]
        # [B,PPAD,3] -> [NCORES,3,NB,PPAD]
        pt = pf.reshape(NCORES, NB, PPAD, 3).transpose(0, 3, 1, 2)
        glhsT[:, 0:3, :, d, :] = -2.0 * pt
        glhsT[:, 3, :, d, :] = 1.0
        glhsT[:, 4, :, d, :] = (pt * pt).sum(axis=1)

        ov = pts[1 - d]                                         # [B,251,3]
        tri = ov[:, faces[1 - d]]                               # [B,500,3,3]
        A, Bv, C = tri[:, :, 0], tri[:, :, 1], tri[:, :, 2]
        n = np.cross(A, Bv) + np.cross(Bv, C) + np.cross(C, A)
        d0 = np.einsum('bfi,bfi->bf', A, np.cross(Bv, C))
        groups = [
            (A,            (A * A).sum(-1),                 1.0),
            (Bv,           (Bv * Bv).sum(-1),               1.0),
            (C,            (C * C).sum(-1),                 1.0),
            ((A + Bv) / 2, np.einsum('bfi,bfi->bf', A, Bv), 1.0),
            ((Bv + C) / 2, np.einsum('bfi,bfi->bf', Bv, C), 1.0),
            ((C + A) / 2,  np.einsum('bfi,bfi->bf', C, A),  1.0),
            (n / 2,        d0,                              0.0),
        ]
        for g, (xyz, c3, ones) in enumerate(groups):
            # [B,500,3] -> [NCORES,3,NB,500]
            xt = xyz.reshape(NCORES, NB, F, 3).transpose(0, 3, 1, 2)
            gfrhs[:, 0:3, :, d, g, :F] = xt
            gfrhs[:, 3, :, d, g, :F] = c3.reshape(NCORES, NB, F)
            gfrhs[:, 4, :, d, g, :F] = ones

        ot = ov.reshape(NCORES, NB, P, 3).transpose(0, 3, 1, 2)
        gmrhs[:, 0:3, :, d, :P] = ot
        gmrhs[:, 3, :, d, :P] = (ot * ot).sum(axis=1)
        gmrhs[:, 4, :, d, :P] = 1.0

    return {
        "lhsT": glhsT.reshape(NCORES * 5, NBD, PPAD),
        "frhs": gfrhs.reshape(NCORES * 5, NBD, 7, 512),
        "mrhs": gmrhs.reshape(NCORES * 5, NBD, PPAD),
    }


# --------------------------------------------------------------------------
# device kernel
# --------------------------------------------------------------------------

def _kernel_body(tc, lhsT_d, frhs_d, mrhs_d, loss_d, dbg=None):
    import concourse.mybir as mybir
    nc = tc.nc
    fp32 = mybir.dt.float32
    AF = mybir.ActivationFunctionType
    OP = mybir.AluOpType
    AX = mybir.AxisListType.X

    with (
        tc.tile_pool(name="const", bufs=1) as cpool,
        tc.tile_pool(name="store", bufs=1) as spool,
        tc.tile_pool(name="stage", bufs=2) as stpool,
        tc.tile_pool(name="iface", bufs=2) as ipool,
        tc.tile_pool(name="dve", bufs=1) as vpool,
    ):
        lhsT_sb = cpool.tile([5, NBD, PPAD], fp32)
        nc.sync.dma_start(lhsT_sb[:], lhsT_d[:])

        ones = cpool.tile([128, 1], fp32)
        nc.vector.memset(ones[:], 1.0)

        sacc = cpool.tile([128, NBLK], fp32)     # per block: sum_f half-angle
        minda = cpool.tile([128, NBLK], fp32)    # per block: clamped min d^2
        denoms = spool.tile([128, SUPER, 512], fp32)
        tts = spool.tile([128, SUPER, 512], fp32)

        def pass_a(ppool, i, j):
            bd, ch = divmod(i, 2)
            if ch == 0:
                fstage = stpool.tile([5, 7, 512], fp32, tag="fstage")
                mstage = stpool.tile([5, PPAD], fp32, tag="mstage")
                nc.sync.dma_start(fstage[:], frhs_d[:, bd])
                nc.sync.dma_start(mstage[:], mrhs_d[:, bd])
                pass_a.stage = (fstage, mstage)
            fstage, mstage = pass_a.stage
            lhs = lhsT_sb[:, bd, ch * 128:(ch + 1) * 128]       # [5,128]

            wind = ppool.tile([128, 7, 512], fp32, tag="wind")
            md = ppool.tile([128, 256], fp32, tag="md")

            for g in range(7):
                nc.tensor.matmul(wind[:, g, :F], lhs, fstage[:, g, :F])
            nc.tensor.matmul(md[:, :P], lhs, mstage[:, :P])

            # min-distance: free-dim min, clamp at 0 (matmul roundoff)
            mind = vpool.tile([128, 1], fp32, tag="mind")
            nc.vector.tensor_reduce(mind[:], md[:, :P], AX, OP.min)
            nc.vector.tensor_scalar(minda[:, i:i + 1], mind[:], 0.0, None,
                                    OP.max)

            # norms: clamp squared lengths at 0 (fp32 matmul roundoff), sqrt
            rl = ipool.tile([128, 3, 512], fp32, tag="rl")
            for g in range(3):
                nc.scalar.activation(rl[:, g, :F], wind[:, g, :F], AF.Relu)
            la = ipool.tile([128, 512], fp32, tag="la")
            lb = ipool.tile([128, 512], fp32, tag="lb")
            lc = ipool.tile([128, 512], fp32, tag="lc")
            nc.scalar.activation(la[:, :F], rl[:, 0, :F], AF.Sqrt)
            nc.scalar.activation(lb[:, :F], rl[:, 1, :F], AF.Sqrt)
            nc.scalar.activation(lc[:, :F], rl[:, 2, :F], AF.Sqrt)
            dets = ipool.tile([128, 512], fp32, tag="dets")
            nc.scalar.activation(dets[:, :F], wind[:, 6, :F], AF.Copy)

            # denominator chain (DVE); PSUM reads scheduled early
            u = vpool.tile([128, 512], fp32, tag="u")
            r4 = vpool.tile([128, 512], fp32, tag="r4")
            s5 = vpool.tile([128, 512], fp32, tag="s5")
            v = vpool.tile([128, 512], fp32, tag="v")
            w = vpool.tile([128, 512], fp32, tag="w")
            t6 = vpool.tile([128, 512], fp32, tag="t6")
            nc.vector.tensor_tensor(r4[:, :F], wind[:, 4, :F], la[:, :F],
                                    OP.mult)
            nc.vector.tensor_tensor(s5[:, :F], wind[:, 5, :F], lb[:, :F],
                                    OP.mult)
            nc.vector.tensor_tensor(u[:, :F], la[:, :F], lb[:, :F], OP.mult)
            nc.vector.tensor_tensor(v[:, :F], u[:, :F], wind[:, 3, :F],
                                    OP.add)

            # rest of the chain is SBUF-only
            w_ = w[:, :F]
            nc.vector.tensor_tensor(w_, v[:, :F], lc[:, :F], OP.mult)
            nc.vector.tensor_tensor(t6[:, :F], r4[:, :F], s5[:, :F], OP.add)
            den = denoms[:, j, :F]
            nc.vector.tensor_tensor(den, w_, t6[:, :F], OP.add)

            # half-angle atan2 range reduction: tt = det / (rho + |den|)
            xx = ipool.tile([128, 512], fp32, tag="xx")
            yy = ipool.tile([128, 512], fp32, tag="yy")
            ss = vpool.tile([128, 512], fp32, tag="ss", bufs=2)
            rho = ipool.tile([128, 512], fp32, tag="rho")
            axd = ipool.tile([128, 512], fp32, tag="axd")
            dd = vpool.tile([128, 512], fp32, tag="dd")
            rd = vpool.tile([128, 512], fp32, tag="rd")
            nc.scalar.activation(xx[:, :F], den, AF.Square)
            nc.scalar.activation(yy[:, :F], dets[:, :F], AF.Square)
            nc.vector.scalar_tensor_tensor(ss[:, :F], xx[:, :F], 1e-20,
                                           yy[:, :F], OP.add, OP.add)
            nc.scalar.activation(rho[:, :F], ss[:, :F], AF.Sqrt)
            nc.scalar.activation(axd[:, :F], den, AF.Abs)
            nc.vector.tensor_tensor(dd[:, :F], rho[:, :F], axd[:, :F], OP.add)
            nc.vector.reciprocal_approx_fast(rd[:, :F], dd[:, :F])
            nc.vector.tensor_tensor(tts[:, j, :F], dets[:, :F], rd[:, :F],
                                    OP.mult)
            if dbg is not None and i == 0:
                wcopy = vpool.tile([128, 7, 512], fp32, tag="wcopy")
                for g in range(7):
                    nc.scalar.activation(wcopy[:, g, :F], wind[:, g, :F], AF.Copy)
                    nc.sync.dma_start(dbg["wind0"][:, g, :F], wcopy[:, g, :F])
                nc.sync.dma_start(dbg["den0"][:, :F], denoms[:, 0, :F])
                nc.sync.dma_start(dbg["tt0"][:, :F], tts[:, 0, :F])

        def pass_b(i, j):
            den = denoms[:, j, :F]
            tt = tts[:, j, :F]
            sgn = ipool.tile([128, 512], fp32, tag="sgn")
            spi = ipool.tile([128, 512], fp32, tag="spi")
            atn = ipool.tile([128, 512], fp32, tag="atn")
            c0 = vpool.tile([128, 512], fp32, tag="c0")
            c1 = vpool.tile([128, 512], fp32, tag="c1")
            sd = vpool.tile([128, 512], fp32, tag="sd")
            nc.scalar.activation(sgn[:, :F], tt, AF.Sign)
            nc.scalar.mul(spi[:, :F], sgn[:, :F], HALF_PI)
            nc.scalar.activation(atn[:, :F], tt, AF.Arctan)
            # half = atn + [den<0]*(pi/2*sign(det) - 2*atn); sign(det)==sign(atn)
            # (gpsimd offload of these was tried: fails in the bass2jax/PJRT
            # lowering, so they stay on DVE)
            nc.vector.scalar_tensor_tensor(c0[:, :F], atn[:, :F], -2.0,
                                           spi[:, :F], OP.mult, OP.add)
            nc.vector.scalar_tensor_tensor(c1[:, :F], den, 0.0,
                                           c0[:, :F], OP.is_lt, OP.mult)
            nc.vector.scalar_tensor_tensor(sd[:, :F], atn[:, :F], 0.0,
                                           c1[:, :F], OP.add, OP.add,
                                           accum_out=sacc[:, i:i + 1])

        with tc.tile_pool(name="psum", bufs=1, space="PSUM") as ppool:
            for s in range(NBLK // SUPER):
                for j in range(SUPER):
                    pass_a(ppool, s * SUPER + j, j)
                tc.no_sync_barrier()
                for j in range(SUPER):
                    pass_b(s * SUPER + j, j)
                tc.no_sync_barrier()

        # ---------------- final: depth * inside, partition-reduce ----------
        inside = cpool.tile([128, NBLK], fp32)
        depth = cpool.tile([128, NBLK], fp32)
        contrib = cpool.tile([128, NBLK], fp32)
        beps = cpool.tile([128, 1], fp32)
        nc.vector.memset(beps[:], 1e-12)
        nc.vector.tensor_scalar(inside[:], sacc[:], HALF_PI, None, OP.is_gt)
        nc.scalar.activation(depth[:], minda[:], AF.Sqrt, bias=beps[:])
        nc.vector.tensor_tensor(contrib[:], depth[:], inside[:], OP.mult)

        with tc.tile_pool(name="psum2", bufs=1, space="PSUM") as p2:
            lpsum = p2.tile([NBLK, 1], fp32)
            nc.tensor.matmul(lpsum[:], contrib[:], ones[:])
            loss_sb = cpool.tile([NBLK, 1], fp32)
            nc.scalar.activation(loss_sb[:], lpsum[:], AF.Copy)
            nc.sync.dma_start(loss_d[:], loss_sb[:])
        if dbg is not None:
            nc.sync.dma_start(dbg["sacc"][:], sacc[:])
            nc.sync.dma_start(dbg["minda"][:], minda[:])


def _build():
    global _compiled
    if _compiled is not None:
        return _compiled
    import concourse.bacc as bacc
    import concourse.mybir as mybir
    import concourse.tile as tile

    nc = bacc.Bacc("TRN2", target_bir_lowering=False, debug=False,
                   num_devices=NCORES)
    fp32 = mybir.dt.float32
    lhsT_d = nc.dram_tensor("lhsT", (5, NBD, PPAD), fp32, kind="ExternalInput").ap()
    frhs_d = nc.dram_tensor("frhs", (5, NBD, 7, 512), fp32, kind="ExternalInput").ap()
    mrhs_d = nc.dram_tensor("mrhs", (5, NBD, PPAD), fp32, kind="ExternalInput").ap()
    loss_d = nc.dram_tensor("loss", (NBLK, 1), fp32, kind="ExternalOutput").ap()

    with tile.TileContext(nc) as tc:
        _kernel_body(tc, lhsT_d, frhs_d, mrhs_d, loss_d)
    nc.compile()
    _compiled = nc
    return nc


# --------------------------------------------------------------------------
# dispatch: jit(shard_map(bass_exec)) built ONCE and cached.
#
# The stock run_bass_kernel_spmd -> run_bass_via_pjrt path creates a fresh
# jax.jit closure on every call, so each kernel() invocation pays a full
# retrace + relower + executable lookup.  Building the jitted callable once
# and reusing it drops per-call overhead to input transfer + execute.
# --------------------------------------------------------------------------

_dispatch = None


def _get_dispatch():
    global _dispatch
    if _dispatch is not None:
        return _dispatch
    import jax
    from jax.experimental.shard_map import shard_map
    from jax.sharding import Mesh, PartitionSpec
    from concourse import bass2jax
    import concourse.mybir as mybir

    nc = _build()
    bass2jax.install_neuronx_cc_hook()
    assert nc.dbg_addr is None
    part_name = nc.partition_id_tensor.name if nc.partition_id_tensor else None

    in_names, out_names, out_avals = [], [], []
    for alloc in nc.m.functions[0].allocations:
        if not isinstance(alloc, mybir.MemoryLocationSet):
            continue
        name = alloc.memorylocations[0].name
        if alloc.kind == "ExternalInput":
            if name != part_name:
                in_names.append(name)
        elif alloc.kind == "ExternalOutput":
            out_names.append(name)
            out_avals.append(jax.core.ShapedArray(
                tuple(alloc.tensor_shape), mybir.dt.np(alloc.dtype)))
    n_params = len(in_names)
    all_names = tuple(in_names) + tuple(out_names)
    if part_name is not None:
        all_names = all_names + (part_name,)

    def _body(*args):
        operands = list(args)
        if part_name is not None:
            operands.append(bass2jax.partition_id_tensor())
        outs = bass2jax._bass_exec_p.bind(
            *operands,
            out_avals=tuple(out_avals),
            in_names=all_names,
            out_names=tuple(out_names),
            lowering_input_output_aliases=(),
            sim_require_finite=True,
            sim_require_nnan=True,
            nc=nc,
        )
        return tuple(outs)

    devices = jax.devices()[:NCORES]
    mesh = Mesh(np.asarray(devices), ("core",))
    in_specs = (PartitionSpec("core"),) * (n_params + len(out_names))
    out_specs = (PartitionSpec("core"),) * len(out_names)
    donate = tuple(range(n_params, n_params + len(out_names)))
    sharded = jax.jit(
        shard_map(_body, mesh=mesh, in_specs=in_specs,
                  out_specs=out_specs, check_rep=False),
        donate_argnums=donate, keep_unused=True)
    shapes = {"lhsT": (NCORES * 5, NBD, PPAD),
              "frhs": (NCORES * 5, NBD, 7, 512),
              "mrhs": (NCORES * 5, NBD, PPAD)}
    _dispatch = (sharded, tuple(in_names), shapes)
    return _dispatch


def _global_feed(lhsT, frhs, mrhs):
    # per-core [5, NBD, ...] stacked on axis 0 -> [NCORES*5, NBD, ...]
    def to_global(a, tail):
        return np.ascontiguousarray(
            a.reshape((NCORES, NBD, 5) + tail).transpose(0, 2, 1, 3)
             .reshape((NCORES * 5, NBD) + tail))
    return {
        "lhsT": to_global(lhsT.reshape(B, 2, 5, PPAD), (PPAD,)),
        "frhs": np.ascontiguousarray(
            frhs.reshape(NCORES, NBD, 5, 7 * 512).transpose(0, 2, 1, 3)
                .reshape(NCORES * 5, NBD, 7, 512)),
        "mrhs": to_global(mrhs.reshape(B, 2, 5, PPAD), (PPAD,)),
    }


def kernel(**inputs) -> np.ndarray:
    global last_exec_time_ns
    lhsT, frhs, mrhs = _host_prep(inputs)

    if bool(int(os.environ.get("HAND_KERNEL_TRACE", "0"))):
        # profiling path: stock spmd runner with NTFF tracing
        from concourse.bass_utils import run_bass_kernel_spmd
        nc = _build()
        maps = []
        for c in range(NCORES):
            bs = slice(c * NB, (c + 1) * NB)
            maps.append({
                "lhsT": lhsT[bs].reshape(NBD, 5, PPAD).transpose(1, 0, 2).copy(),
                "frhs": frhs[bs].reshape(NBD, 5, 7, 512).transpose(1, 0, 2, 3).copy(),
                "mrhs": mrhs[bs].reshape(NBD, 5, PPAD).transpose(1, 0, 2).copy(),
            })
        res = run_bass_kernel_spmd(nc, maps, list(range(NCORES)), trace=True)
        last_exec_time_ns = res.exec_time_ns
        loss = np.zeros(B, np.float32)
        for c in range(NCORES):
            out = np.asarray(res.results[c]["loss"], np.float32).reshape(NBLK)
            loss[c * NB:(c + 1) * NB] = out.reshape(NB, 4).sum(axis=1)
        return loss

    sharded, in_names, _shapes = _get_dispatch()
    feed = _global_feed(lhsT, frhs, mrhs)
    (out,) = sharded(*[feed[n] for n in in_names],
                     np.zeros((NCORES * NBLK, 1), np.float32))
    last_exec_time_ns = None
    # block i = (b_loc*2 + dir)*2 + chunk
    return np.asarray(out, np.float32).reshape(B, 4).sum(axis=1)



# revision 25
# speedup vs baseline: 6.2980x; 3.8629x over previous
"""Trainium2 Bass kernel for nn_HandIntersectionLoss.

Strategy
--------
Pure data parallel over batch: 64 batches -> 8 cores x 8 local batches.

The reference math is reformulated so the tensor engine does the heavy
per-(point, face) lifting via K=5 matmuls (polynomial expansion of the
Van Oosterom / Strackee solid-angle terms):

    |A-p|^2          = |A|^2 - 2 p.A + |p|^2
    (A-p).(B-p)      = |m-p|^2 - |A-B|^2/2,  m = (A+B)/2   (polarization)
    det(A-p,B-p,C-p) = A.(BxC) - p.(AxB + BxC + CxA)

With moving rows [-2px,-2py,-2pz, 1, |p|^2] a single matmul against
per-face constant columns produces la^2, lb^2, lc^2, ab, bc, ca, det
for a [128 points x 500 faces] block.

The call is made over a high-latency axon tunnel (~75ms RTT, ~100MB/s),
so the per-face constant tensors are constructed ON DEVICE from a tiny
upload (the moving-point rows + face indices as floats + small constant
matrices).  Device-side construction:

  1. mrhs (other-hand vertex rows [x,y,z,|v|^2,1]) = 5x5 row-mix matmul
     of the uploaded moving rows of the opposite hand.
  2. V5T (vertex table transposed, [vert, (batch,коord)]) = PE transposes.
  3. One-hot gather matrices from face indices: broadcast face row via
     K=1 matmul, compare against an uploaded iota column (is_equal).
  4. Corner groups 0-2: V5T^T @ onehot  (batched over 4 batches per
     matmul: output partitions (nb%4)*32 + k).
  5. Mid groups 3-5: 0.5*(gA+gB) + scatter(-|A-B|^2/2) where the scatter
     is a matmul with a shifted slice of an uploaded selection master.
  6. Group 6: scatter matmuls of DVE-computed n/2 = (AxB+BxC+CxA)/2 and
     d0 = A.(BxC) rows.

The per-element chain (denominator assembly + range-reduced atan2) runs
on DVE/ACT exactly as before:

    atan2(det, den) = 2*atan(det / (rho + |den|))            (den >= 0)
                    = sign(det)*pi - 2*atan(det/(rho+|den|)) (den < 0)
    rho = sqrt(det^2 + den^2 + 1e-20)   -> |atan input| <= 1 always

inside(p) <=> sum_f half > pi/2.  Min-distance uses the same matmul
trick + free-dim min-reduce.  Scalar-engine table sets force a two-pass
structure (sqrt and arctan live in different ACT table sets).

Dispatch: jit(shard_map(bass_exec)) built ONCE and cached -- the stock
run_bass_kernel_spmd path creates a fresh jax.jit closure per call and
pays a full retrace every time.
"""
import os
import sys
import numpy as np

sys.path.insert(0, '/opt/trn_rl_repo')

B, V_FULL, V_HAND, V_LOOP, N_FACES = 64, 6890, 250, 20, 500
P = V_HAND + 1          # 251 points/verts per hand (incl. lid)
PPAD = 256
NCORES = 8
NB = B // NCORES        # local batches per core
NBD = NB * 2            # (batch, dir) pairs per core
NBLK = NBD * 2          # blocks per core: x2 point-chunks of 128
SUPER = 8               # blocks per two-pass super-group
F = N_FACES
HALF_PI = float(np.pi / 2)

_compiled = None        # cached compiled program across kernel() calls
_dispatch = None
last_exec_time_ns = None


# --------------------------------------------------------------------------
# host prep: tiny uploads only
# --------------------------------------------------------------------------

def _host_prep2(inputs):
    verts = np.asarray(inputs['verts_batch'], dtype=np.float32)

    pts = {}
    for d, (hi, li) in enumerate([
            ('hand_verts_inds_left', 'hand_loop_verts_inds_left'),
            ('hand_verts_inds_right', 'hand_loop_verts_inds_right')]):
        h = verts[:, np.asarray(inputs[hi])]                    # [B,250,3]
        lid = verts[:, np.asarray(inputs[li])].mean(
            axis=1, keepdims=True, dtype=np.float32)
        pts[d] = np.concatenate([h, lid], axis=1)               # [B,251,3]

    # rows [x, y, z, |p|^2]; the -2 scale and the ones row are added on device
    gpts = np.full((NCORES, 4, NB, 2, PPAD), 1e3, np.float32)
    gpts[:, 3] = 3e6
    for d in range(2):
        pr = pts[d].reshape(NCORES, NB, P, 3).transpose(0, 3, 1, 2)
        gpts[:, 0:3, :, d, :P] = pr
        gpts[:, 3, :, d, :P] = (pr * pr).sum(axis=1)

    # faces of the OTHER hand per dir, corner-major, as floats (pad 300)
    facesf = np.full((2, 3, 512), 300.0, np.float32)
    of = {0: np.asarray(inputs['hand_faces_right']),
          1: np.asarray(inputs['hand_faces_left'])}
    for d in range(2):
        facesf[d, :, :F] = of[d].T.astype(np.float32)

    # selection master: P[k,h] = master[:, 128-k+128h : 256-k+128h]
    master = np.zeros((8, 384), np.float32)
    for nb in range(8):
        master[nb, nb * 32 + 128] = 1.0

    iota2 = (np.arange(128, dtype=np.float32)[:, None]
             + np.array([0.0, 128.0], np.float32)[None, :])     # [128,2]

    pmm = np.zeros((5, 16), np.float32)
    pmm[0, 0] = pmm[1, 1] = pmm[2, 2] = -0.5    # M5T cols 0-4
    pmm[4, 3] = 1.0
    pmm[3, 4] = 1.0
    for k in range(5):
        pmm[k, 5 + k] = 1.0                     # I5 cols 5-9

    return {
        "ptsT": gpts.reshape(NCORES * 4, NBD, PPAD),
        "faces": np.tile(facesf.reshape(1, 3072), (NCORES, 1)),
        "master": np.tile(master, (NCORES, 1)),
        "iota": np.tile(iota2, (NCORES, 1)),
        "pmm": np.tile(pmm, (NCORES, 1)),
    }


# --------------------------------------------------------------------------
# device kernel
# --------------------------------------------------------------------------

def _kernel_body(tc, ptsT_d, faces_d, master_d, iota_d, pmm_d, loss_d,
                 dbg=None):
    import concourse.mybir as mybir
    nc = tc.nc
    fp32 = mybir.dt.float32
    AF = mybir.ActivationFunctionType
    OP = mybir.AluOpType
    AX = mybir.AxisListType.X

    with tc.tile_pool(name="const", bufs=1) as cpool:
        # ---- persistent tiles --------------------------------------------
        # lhsT4: moving rows [-2x,-2y,-2z,1,|p|^2] built from raw xyz,
        # replicated at the four 32-partition offsets.
        lhsT4 = cpool.tile([128, NBD, PPAD], fp32)
        with tc.tile_pool(name="lhsTbuild", bufs=1) as lbp:
            onesrow = lbp.tile([1, NBD, PPAD], fp32, tag="onesrow")
            nc.vector.memset(onesrow[:], 1.0)
            for j in range(4):
                nc.sync.dma_start(lhsT4[32 * j:32 * j + 3], ptsT_d[0:3])
                nc.sync.dma_start(lhsT4[32 * j + 4:32 * j + 5], ptsT_d[3:4])
                nc.sync.dma_start(lhsT4[32 * j + 3:32 * j + 4], onesrow[:])
                nc.vector.tensor_scalar(lhsT4[32 * j:32 * j + 3],
                                        lhsT4[32 * j:32 * j + 3],
                                        -2.0, None, OP.mult)
        mrhs4 = cpool.tile([128, NBD, PPAD], fp32)
        # winding rhs group tiles [128, 512], partition (nb%4)*32+k
        gsb = [[[cpool.tile([128, 512], fp32, tag=f"g{d}{c}{h}", name=f"g{d}{c}{h}")
                 for h in range(2)] for c in range(3)] for d in range(2)]
        midsb = [[[cpool.tile([128, 512], fp32, tag=f"m{d}{gi}{h}", name=f"m{d}{gi}{h}")
                   for h in range(2)] for gi in range(3)] for d in range(2)]
        g6sb = [[cpool.tile([128, 512], fp32, tag=f"s{d}{h}", name=f"s{d}{h}")
                 for h in range(2)] for d in range(2)]
        ones = cpool.tile([128, 1], fp32)
        nc.vector.memset(ones[:], 1.0)
        sacc = cpool.tile([128, NBLK], fp32)     # per block: sum_f half-angle
        minda = cpool.tile([128, NBLK], fp32)    # per block: clamped min d^2

        # =============== construction phase ===============================
        with (
            tc.tile_pool(name="prep", bufs=1) as prp,
            tc.tile_pool(name="prept", bufs=2) as prt,
            tc.tile_pool(name="preps", bufs=1, space="PSUM") as pps,
        ):
            faces_sb = prp.tile([1, 3072], fp32, tag="faces")
            nc.sync.dma_start(faces_sb[:], faces_d[:])
            master_sb = prp.tile([8, 384], fp32, tag="master")
            nc.sync.dma_start(master_sb[:], master_d[:])
            iota_sb = prp.tile([128, 2], fp32, tag="iota")
            nc.sync.dma_start(iota_sb[:], iota_d[:])
            pmm_sb = prp.tile([5, 16], fp32, tag="pmm")
            nc.sync.dma_start(pmm_sb[:], pmm_d[:])
            ones1 = prp.tile([1, 128], fp32, tag="ones1")
            nc.vector.memset(ones1[:], 1.0)

            # S1: mrhs4 (other-hand vertex rows) via M5T row-mix, at the
            # four 32-partition offsets needed by the per-batch matmuls.
            for bd in range(NBD):
                nb, d = divmod(bd, 2)
                src = lhsT4[0:5, nb * 2 + (1 - d), :]
                ps = pps.tile([128, PPAD], fp32, tag="mrps")
                for j in range(4):
                    nc.tensor.matmul(ps[32 * j:32 * j + 5, :],
                                     pmm_sb[0:5, 0:5], src,
                                     tile_position=(0, 32 * j))
                for j in range(4):
                    nc.scalar.activation(mrhs4[32 * j:32 * j + 5, bd, :],
                                         ps[32 * j:32 * j + 5, :], AF.Copy)

            # S2: V5T[d][ch] [vert(128), nb*32+k] via PE transposes
            v5t = [[prp.tile([128, 256], fp32, tag=f"v{d}{ch}", name=f"v{d}{ch}")
                    for ch in range(2)] for d in range(2)]
            for d in range(2):
                for ch in range(2):
                    ps = pps.tile([128, 256], fp32, tag="v5ps")
                    for nb in range(NB):
                        bd = nb * 2 + d
                        nc.tensor.matmul(
                            ps[:, nb * 32:nb * 32 + 5],
                            mrhs4[0:5, bd, ch * 128:(ch + 1) * 128],
                            pmm_sb[0:5, 5:10])
                    nc.vector.memset(v5t[d][ch][:], 0.0)
                    for nb in range(NB):
                        nc.scalar.activation(
                            v5t[d][ch][:, nb * 32:nb * 32 + 5],
                            ps[:, nb * 32:nb * 32 + 5], AF.Copy)

            # S3-S5a under a scoped one-hot pool; coord outlives it
            with tc.tile_pool(name="coordp", bufs=1) as cop:
                coord = [[[cop.tile([8, 512], fp32, tag=f"c{d}{c}{k}",
                                    name=f"c{d}{c}{k}")
                           for k in range(3)] for c in range(3)]
                         for d in range(2)]
                with tc.tile_pool(name="ohp", bufs=1) as ohp:
                    # S3: one-hot gather matrices oh[d][c][ch]
                    oh = [[[ohp.tile([128, 512], fp32, tag=f"o{d}{c}{ch}",
                                     name=f"o{d}{c}{ch}")
                            for ch in range(2)] for c in range(3)]
                          for d in range(2)]
                    for d in range(2):
                        for c in range(3):
                            ps = pps.tile([128, 512], fp32, tag="fbps")
                            nc.tensor.matmul(
                                ps[:], ones1[0:1, :],
                                faces_sb[0:1, (d * 3 + c) * 512:
                                         (d * 3 + c + 1) * 512])
                            for ch in range(2):
                                nc.vector.tensor_scalar(
                                    oh[d][c][ch][:], ps[:],
                                    iota_sb[:, ch:ch + 1], None,
                                    OP.is_equal)

                    # S4: corner groups 0-2 (also reused for mids)
                    for d in range(2):
                        for c in range(3):
                            for h in range(2):
                                ps = pps.tile([128, 512], fp32, tag="gps")
                                nc.tensor.matmul(
                                    ps[:, :F],
                                    v5t[d][0][:, h * 128:(h + 1) * 128],
                                    oh[d][c][0][:, :F],
                                    start=True, stop=False)
                                nc.tensor.matmul(
                                    ps[:, :F],
                                    v5t[d][1][:, h * 128:(h + 1) * 128],
                                    oh[d][c][1][:, :F],
                                    start=False, stop=True)
                                nc.scalar.activation(gsb[d][c][h][:, :F],
                                                     ps[:, :F], AF.Copy)

                    # S5a: coord gathers for BOTH dirs
                    for d in range(2):
                        for c in range(3):
                            for k in range(3):
                                ps8 = pps.tile([8, 512], fp32, tag="cps")
                                nc.tensor.matmul(
                                    ps8[:, :F], v5t[d][0][:, k::32],
                                    oh[d][c][0][:, :F],
                                    start=True, stop=False)
                                nc.tensor.matmul(
                                    ps8[:, :F], v5t[d][1][:, k::32],
                                    oh[d][c][1][:, :F],
                                    start=False, stop=True)
                                nc.scalar.activation(coord[d][c][k][:, :F],
                                                     ps8[:, :F], AF.Copy)

                # S5b-S7 per dir: bilinear chain, mids, group 6
                for d in range(2):
                    with (
                        tc.tile_pool(name=f"chain{d}", bufs=1) as chp,
                        tc.tile_pool(name=f"chaint{d}", bufs=2) as cht,
                    ):
                        A, Bv, C = coord[d]
                        # bxc (kept for d0), n/2 rows accumulated in place
                        bxc = [chp.tile([8, 512], fp32, tag=f"b{k}",
                                        name=f"b{k}") for k in range(3)]
                        nh = [chp.tile([8, 512], fp32, tag=f"n{k}",
                                       name=f"n{k}") for k in range(3)]

                        def crossk(U, V, out, k):
                            k1, k2 = (k + 1) % 3, (k + 2) % 3
                            p1 = cht.tile([8, 512], fp32, tag="p1")
                            p2 = cht.tile([8, 512], fp32, tag="p2")
                            nc.vector.tensor_tensor(
                                p1[:, :F], U[k1][:, :F], V[k2][:, :F],
                                OP.mult)
                            nc.vector.tensor_tensor(
                                p2[:, :F], U[k2][:, :F], V[k1][:, :F],
                                OP.mult)
                            nc.vector.tensor_tensor(
                                out[:, :F], p1[:, :F], p2[:, :F],
                                OP.subtract)

                        for k in range(3):
                            crossk(Bv, C, bxc[k], k)
                            t = cht.tile([8, 512], fp32, tag="nt")
                            u = cht.tile([8, 512], fp32, tag="nu")
                            crossk(A, Bv, t, k)             # axb_k
                            crossk(C, A, u, k)              # cxa_k
                            nc.vector.tensor_tensor(t[:, :F], t[:, :F],
                                                    bxc[k][:, :F], OP.add)
                            nc.vector.tensor_tensor(t[:, :F], t[:, :F],
                                                    u[:, :F], OP.add)
                            nc.vector.tensor_scalar(nh[k][:, :F], t[:, :F],
                                                    0.5, None, OP.mult)
                        # d0 = A.(BxC)
                        d0 = chp.tile([8, 512], fp32, tag="d0")
                        t = cht.tile([8, 512], fp32, tag="nt")
                        nc.vector.tensor_tensor(d0[:, :F], A[0][:, :F],
                                                bxc[0][:, :F], OP.mult)
                        nc.vector.tensor_tensor(t[:, :F], A[1][:, :F],
                                                bxc[1][:, :F], OP.mult)
                        nc.vector.tensor_tensor(d0[:, :F], d0[:, :F],
                                                t[:, :F], OP.add)
                        nc.vector.tensor_tensor(t[:, :F], A[2][:, :F],
                                                bxc[2][:, :F], OP.mult)
                        nc.vector.tensor_tensor(d0[:, :F], d0[:, :F],
                                                t[:, :F], OP.add)
                        # em[pair] = -0.5*|U-V|^2
                        pairs = [(A, Bv), (Bv, C), (C, A)]
                        em = [chp.tile([8, 512], fp32, tag=f"e{gi}",
                                       name=f"e{gi}") for gi in range(3)]
                        for gi, (U, V) in enumerate(pairs):
                            acc = cht.tile([8, 512], fp32, tag="p1")
                            for k in range(3):
                                s = cht.tile([8, 512], fp32, tag="p2")
                                nc.vector.tensor_tensor(
                                    s[:, :F], U[k][:, :F], V[k][:, :F],
                                    OP.subtract)
                                if k == 0:
                                    nc.vector.tensor_tensor(
                                        acc[:, :F], s[:, :F], s[:, :F],
                                        OP.mult)
                                else:
                                    sq = cht.tile([8, 512], fp32, tag="nu")
                                    nc.vector.tensor_tensor(
                                        sq[:, :F], s[:, :F], s[:, :F],
                                        OP.mult)
                                    nc.vector.tensor_tensor(
                                        acc[:, :F], acc[:, :F], sq[:, :F],
                                        OP.add)
                            nc.vector.tensor_scalar(em[gi][:, :F],
                                                    acc[:, :F],
                                                    -0.5, None, OP.mult)

                        # S6: mid groups 3-5 = 0.5*(gA+gB) + scatter(em)
                        cpairs = [(0, 1), (1, 2), (2, 0)]
                        for gi, (ca, cb) in enumerate(cpairs):
                            for h in range(2):
                                ps = pps.tile([128, 512], fp32,
                                              tag="corrps")
                                off = 125 + 128 * h   # k=3 slice of master
                                nc.tensor.matmul(
                                    ps[:, :F],
                                    master_sb[:, off:off + 128],
                                    em[gi][:, :F])
                                t1 = cht.tile([128, 512], fp32, tag="midt")
                                nc.vector.tensor_tensor(
                                    t1[:, :F], gsb[d][ca][h][:, :F],
                                    gsb[d][cb][h][:, :F], OP.add)
                                nc.vector.scalar_tensor_tensor(
                                    midsb[d][gi][h][:, :F], t1[:, :F], 0.5,
                                    ps[:, :F], OP.mult, OP.add)

                        # S7: group 6 = scatter(n/2 rows) + scatter(d0)
                        for h in range(2):
                            ps = pps.tile([128, 512], fp32, tag="g6ps")
                            for k in range(3):
                                off = 128 - k + 128 * h
                                nc.tensor.matmul(
                                    ps[:, :F],
                                    master_sb[:, off:off + 128],
                                    nh[k][:, :F],
                                    start=(k == 0), stop=False)
                            off = 125 + 128 * h
                            nc.tensor.matmul(ps[:, :F],
                                             master_sb[:, off:off + 128],
                                             d0[:, :F], start=False,
                                             stop=True)
                            nc.scalar.activation(g6sb[d][h][:, :F],
                                                 ps[:, :F], AF.Copy)

        if dbg is not None:
            for d in range(2):
                for c in range(3):
                    for h in range(2):
                        nc.sync.dma_start(dbg[f"g{d}{c}{h}"], gsb[d][c][h][:, :F])
                for gi in range(3):
                    for h in range(2):
                        nc.sync.dma_start(dbg[f"m{d}{gi}{h}"],
                                          midsb[d][gi][h][:, :F])
                for h in range(2):
                    nc.sync.dma_start(dbg[f"s{d}{h}"], g6sb[d][h][:, :F])
            nc.sync.dma_start(dbg["mrhs4"], mrhs4[0:5])
        dbg_post = dbg

        # =============== main winding / min-dist loop =====================
        with (
            tc.tile_pool(name="store", bufs=1) as spool,
            tc.tile_pool(name="iface", bufs=2) as ipool,
            tc.tile_pool(name="dve", bufs=1) as vpool,
        ):
            denoms = spool.tile([128, SUPER, 512], fp32)
            tts = spool.tile([128, SUPER, 512], fp32)

            def pass_a(ppool, i, j):
                bd, ch = divmod(i, 2)
                nb, d = divmod(bd, 2)
                h, q = divmod(nb, 4)
                lhs = lhsT4[q * 32:q * 32 + 5, bd, ch * 128:(ch + 1) * 128]
                G = [gsb[d][0][h], gsb[d][1][h], gsb[d][2][h],
                     midsb[d][0][h], midsb[d][1][h], midsb[d][2][h],
                     g6sb[d][h]]

                wind = ppool.tile([128, 7, 512], fp32, tag="wind")
                md = ppool.tile([128, 256], fp32, tag="md")

                for g in range(7):
                    nc.tensor.matmul(wind[:, g, :F], lhs,
                                     G[g][q * 32:q * 32 + 5, :F],
                                     tile_position=(q * 32, 0))
                nc.tensor.matmul(md[:, :P], lhs,
                                 mrhs4[q * 32:q * 32 + 5, bd, :P],
                                 tile_position=(q * 32, 0))

                # min-distance: free-dim min, clamp at 0 (matmul roundoff)
                mind = vpool.tile([128, 1], fp32, tag="mind")
                nc.vector.tensor_reduce(mind[:], md[:, :P], AX, OP.min)
                nc.vector.tensor_scalar(minda[:, i:i + 1], mind[:], 0.0,
                                        None, OP.max)

                # norms: clamp squared lengths at 0, sqrt
                rl = ipool.tile([128, 3, 512], fp32, tag="rl")
                for g in range(3):
                    nc.scalar.activation(rl[:, g, :F], wind[:, g, :F],
                                         AF.Relu)
                la = ipool.tile([128, 512], fp32, tag="la")
                lb = ipool.tile([128, 512], fp32, tag="lb")
                lc = ipool.tile([128, 512], fp32, tag="lc")
                nc.scalar.activation(la[:, :F], rl[:, 0, :F], AF.Sqrt)
                nc.scalar.activation(lb[:, :F], rl[:, 1, :F], AF.Sqrt)
                nc.scalar.activation(lc[:, :F], rl[:, 2, :F], AF.Sqrt)
                dets = ipool.tile([128, 512], fp32, tag="dets")
                nc.scalar.activation(dets[:, :F], wind[:, 6, :F], AF.Copy)

                # denominator chain (DVE); PSUM reads scheduled early
                u = vpool.tile([128, 512], fp32, tag="u")
                r4 = vpool.tile([128, 512], fp32, tag="r4")
                s5 = vpool.tile([128, 512], fp32, tag="s5")
                v = vpool.tile([128, 512], fp32, tag="v")
                w = vpool.tile([128, 512], fp32, tag="w")
                t6 = vpool.tile([128, 512], fp32, tag="t6")
                nc.vector.tensor_tensor(r4[:, :F], wind[:, 4, :F],
                                        la[:, :F], OP.mult)
                nc.vector.tensor_tensor(s5[:, :F], wind[:, 5, :F],
                                        lb[:, :F], OP.mult)
                nc.vector.tensor_tensor(u[:, :F], la[:, :F], lb[:, :F],
                                        OP.mult)
                nc.vector.tensor_tensor(v[:, :F], u[:, :F], wind[:, 3, :F],
                                        OP.add)

                # rest of the chain is SBUF-only
                w_ = w[:, :F]
                nc.vector.tensor_tensor(w_, v[:, :F], lc[:, :F], OP.mult)
                nc.vector.tensor_tensor(t6[:, :F], r4[:, :F], s5[:, :F],
                                        OP.add)
                den = denoms[:, j, :F]
                nc.vector.tensor_tensor(den, w_, t6[:, :F], OP.add)

                # half-angle atan2 range reduction: tt = det / (rho + |den|)
                xx = ipool.tile([128, 512], fp32, tag="xx")
                yy = ipool.tile([128, 512], fp32, tag="yy")
                ss = vpool.tile([128, 512], fp32, tag="ss", bufs=2)
                rho = ipool.tile([128, 512], fp32, tag="rho")
                axd = ipool.tile([128, 512], fp32, tag="axd")
                dd = vpool.tile([128, 512], fp32, tag="dd")
                rd = vpool.tile([128, 512], fp32, tag="rd")
                nc.scalar.activation(xx[:, :F], den, AF.Square)
                nc.scalar.activation(yy[:, :F], dets[:, :F], AF.Square)
                nc.vector.scalar_tensor_tensor(ss[:, :F], xx[:, :F], 1e-20,
                                               yy[:, :F], OP.add, OP.add)
                nc.scalar.activation(rho[:, :F], ss[:, :F], AF.Sqrt)
                nc.scalar.activation(axd[:, :F], den, AF.Abs)
                nc.vector.tensor_tensor(dd[:, :F], rho[:, :F], axd[:, :F],
                                        OP.add)
                nc.vector.reciprocal_approx_fast(rd[:, :F], dd[:, :F])
                nc.vector.tensor_tensor(tts[:, j, :F], dets[:, :F],
                                        rd[:, :F], OP.mult)

            def pass_b(i, j):
                den = denoms[:, j, :F]
                tt = tts[:, j, :F]
                sgn = ipool.tile([128, 512], fp32, tag="sgn")
                spi = ipool.tile([128, 512], fp32, tag="spi")
                atn = ipool.tile([128, 512], fp32, tag="atn")
                c0 = vpool.tile([128, 512], fp32, tag="c0")
                c1 = vpool.tile([128, 512], fp32, tag="c1")
                sd = vpool.tile([128, 512], fp32, tag="sd")
                nc.scalar.activation(sgn[:, :F], tt, AF.Sign)
                nc.scalar.mul(spi[:, :F], sgn[:, :F], HALF_PI)
                nc.scalar.activation(atn[:, :F], tt, AF.Arctan)
                # half = atn + [den<0]*(pi/2*sign(det) - 2*atn)
                nc.vector.scalar_tensor_tensor(c0[:, :F], atn[:, :F], -2.0,
                                               spi[:, :F], OP.mult, OP.add)
                nc.vector.scalar_tensor_tensor(c1[:, :F], den, 0.0,
                                               c0[:, :F], OP.is_lt, OP.mult)
                nc.vector.scalar_tensor_tensor(sd[:, :F], atn[:, :F], 0.0,
                                               c1[:, :F], OP.add, OP.add,
                                               accum_out=sacc[:, i:i + 1])

            with tc.tile_pool(name="psum", bufs=1, space="PSUM") as ppool:
                for s in range(NBLK // SUPER):
                    for j in range(SUPER):
                        pass_a(ppool, s * SUPER + j, j)
                    tc.no_sync_barrier()
                    for j in range(SUPER):
                        pass_b(s * SUPER + j, j)
                    tc.no_sync_barrier()

            if dbg is not None:
                nc.sync.dma_start(dbg["sacc"], sacc[:])
                nc.sync.dma_start(dbg["minda"], minda[:])

            # ---------------- final: depth * inside, partition-reduce -----
            inside = cpool.tile([128, NBLK], fp32)
            depth = cpool.tile([128, NBLK], fp32)
            contrib = cpool.tile([128, NBLK], fp32)
            beps = cpool.tile([128, 1], fp32)
            nc.vector.memset(beps[:], 1e-12)
            nc.vector.tensor_scalar(inside[:], sacc[:], HALF_PI, None,
                                    OP.is_gt)
            nc.scalar.activation(depth[:], minda[:], AF.Sqrt, bias=beps[:])
            nc.vector.tensor_tensor(contrib[:], depth[:], inside[:],
                                    OP.mult)

            with tc.tile_pool(name="psum2", bufs=1, space="PSUM") as p2:
                lpsum = p2.tile([NBLK, 1], fp32)
                nc.tensor.matmul(lpsum[:], contrib[:], ones[:])
                loss_sb = cpool.tile([NBLK, 1], fp32)
                nc.scalar.activation(loss_sb[:], lpsum[:], AF.Copy)
                nc.sync.dma_start(loss_d[:], loss_sb[:])


def _build():
    global _compiled
    if _compiled is not None:
        return _compiled
    import concourse.bacc as bacc
    import concourse.mybir as mybir
    import concourse.tile as tile

    nc = bacc.Bacc("TRN2", target_bir_lowering=False, debug=False,
                   num_devices=NCORES)
    fp32 = mybir.dt.float32
    ptsT_d = nc.dram_tensor("ptsT", (4, NBD, PPAD), fp32,
                            kind="ExternalInput").ap()
    faces_d = nc.dram_tensor("faces", (1, 3072), fp32,
                             kind="ExternalInput").ap()
    master_d = nc.dram_tensor("master", (8, 384), fp32,
                              kind="ExternalInput").ap()
    iota_d = nc.dram_tensor("iota", (128, 2), fp32,
                            kind="ExternalInput").ap()
    pmm_d = nc.dram_tensor("pmm", (5, 16), fp32, kind="ExternalInput").ap()
    loss_d = nc.dram_tensor("loss", (NBLK, 1), fp32,
                            kind="ExternalOutput").ap()

    with tile.TileContext(nc) as tc:
        _kernel_body(tc, ptsT_d, faces_d, master_d, iota_d, pmm_d, loss_d)
    nc.compile()
    _compiled = nc
    return nc


# --------------------------------------------------------------------------
# dispatch: jit(shard_map(bass_exec)) built ONCE and cached.
# --------------------------------------------------------------------------

def _get_dispatch():
    global _dispatch
    if _dispatch is not None:
        return _dispatch
    import jax
    from jax.experimental.shard_map import shard_map
    from jax.sharding import Mesh, PartitionSpec
    from concourse import bass2jax
    import concourse.mybir as mybir

    nc = _build()
    bass2jax.install_neuronx_cc_hook()
    assert nc.dbg_addr is None
    part_name = nc.partition_id_tensor.name if nc.partition_id_tensor else None

    in_names, out_names, out_avals = [], [], []
    for alloc in nc.m.functions[0].allocations:
        if not isinstance(alloc, mybir.MemoryLocationSet):
            continue
        name = alloc.memorylocations[0].name
        if alloc.kind == "ExternalInput":
            if name != part_name:
                in_names.append(name)
        elif alloc.kind == "ExternalOutput":
            out_names.append(name)
            out_avals.append(jax.core.ShapedArray(
                tuple(alloc.tensor_shape), mybir.dt.np(alloc.dtype)))
    n_params = len(in_names)
    all_names = tuple(in_names) + tuple(out_names)
    if part_name is not None:
        all_names = all_names + (part_name,)

    def _body(*args):
        operands = list(args)
        if part_name is not None:
            operands.append(bass2jax.partition_id_tensor())
        outs = bass2jax._bass_exec_p.bind(
            *operands,
            out_avals=tuple(out_avals),
            in_names=all_names,
            out_names=tuple(out_names),
            lowering_input_output_aliases=(),
            sim_require_finite=True,
            sim_require_nnan=True,
            nc=nc,
        )
        return tuple(outs)

    devices = jax.devices()[:NCORES]
    mesh = Mesh(np.asarray(devices), ("core",))
    in_specs = (PartitionSpec("core"),) * (n_params + len(out_names))
    out_specs = (PartitionSpec("core"),) * len(out_names)
    donate = tuple(range(n_params, n_params + len(out_names)))
    sharded = jax.jit(
        shard_map(_body, mesh=mesh, in_specs=in_specs,
                  out_specs=out_specs, check_rep=False),
        donate_argnums=donate, keep_unused=True)
    _dispatch = (sharded, tuple(in_names))
    return _dispatch


# --------------------------------------------------------------------------
# entry point
# --------------------------------------------------------------------------

def kernel(**inputs) -> np.ndarray:
    global last_exec_time_ns
    feed = _host_prep2(inputs)

    if bool(int(os.environ.get("HAND_KERNEL_TRACE", "0"))):
        # profiling path: stock spmd runner with NTFF tracing
        from concourse.bass_utils import run_bass_kernel_spmd
        nc = _build()
        percore = {"ptsT": (NCORES, 4), "faces": (NCORES, 1),
                   "master": (NCORES, 8), "iota": (NCORES, 128),
                   "pmm": (NCORES, 5)}
        maps = []
        for c in range(NCORES):
            m = {}
            for k, (nco, p0) in percore.items():
                g = feed[k]
                m[k] = g.reshape((nco, p0) + g.shape[1:])[c].copy()
            maps.append(m)
        res = run_bass_kernel_spmd(nc, maps, list(range(NCORES)), trace=True)
        last_exec_time_ns = res.exec_time_ns
        loss = np.zeros(B, np.float32)
        for c in range(NCORES):
            out = np.asarray(res.results[c]["loss"], np.float32).reshape(NBLK)
            loss[c * NB:(c + 1) * NB] = out.reshape(NB, 4).sum(axis=1)
        return loss

    sharded, in_names = _get_dispatch()
    (out,) = sharded(*[feed[n] for n in in_names],
                     np.zeros((NCORES * NBLK, 1), np.float32))
    last_exec_time_ns = None
    # block i = ((b_loc*2 + dir)*2 + chunk)
    return np.asarray(out, np.float32).reshape(B, 4).sum(axis=1)


# revision 26
# speedup vs baseline: 7.0276x; 1.1159x over previous
"""Trainium2 Bass kernel for nn_HandIntersectionLoss.

Strategy
--------
Pure data parallel over batch: 64 batches -> 8 cores x 8 local batches.

The reference math is reformulated so the tensor engine does the heavy
per-(point, face) lifting via K=5 matmuls (polynomial expansion of the
Van Oosterom / Strackee solid-angle terms):

    |A-p|^2          = |A|^2 - 2 p.A + |p|^2
    (A-p).(B-p)      = |m-p|^2 - |A-B|^2/2,  m = (A+B)/2   (polarization)
    det(A-p,B-p,C-p) = A.(BxC) - p.(AxB + BxC + CxA)

With moving rows [-2px,-2py,-2pz, 1, |p|^2] a single matmul against
per-face constant columns produces la^2, lb^2, lc^2, ab, bc, ca, det
for a [128 points x 500 faces] block.

The call is made over a high-latency axon tunnel (~75ms RTT, ~100MB/s),
so the per-face constant tensors are constructed ON DEVICE from a tiny
upload (the moving-point rows + face indices as floats + small constant
matrices).  Device-side construction:

  1. mrhs (other-hand vertex rows [x,y,z,|v|^2,1]) = 5x5 row-mix matmul
     of the uploaded moving rows of the opposite hand.
  2. V5T (vertex table transposed, [vert, (batch,коord)]) = PE transposes.
  3. One-hot gather matrices from face indices: broadcast face row via
     K=1 matmul, compare against an uploaded iota column (is_equal).
  4. Corner groups 0-2: V5T^T @ onehot  (batched over 4 batches per
     matmul: output partitions (nb%4)*32 + k).
  5. Mid groups 3-5: 0.5*(gA+gB) + scatter(-|A-B|^2/2) where the scatter
     is a matmul with a shifted slice of an uploaded selection master.
  6. Group 6: scatter matmuls of DVE-computed n/2 = (AxB+BxC+CxA)/2 and
     d0 = A.(BxC) rows.

The per-element chain (denominator assembly + range-reduced atan2) runs
on DVE/ACT exactly as before:

    atan2(det, den) = 2*atan(det / (rho + |den|))            (den >= 0)
                    = sign(det)*pi - 2*atan(det/(rho+|den|)) (den < 0)
    rho = sqrt(det^2 + den^2 + 1e-20)   -> |atan input| <= 1 always

inside(p) <=> sum_f half > pi/2.  Min-distance uses the same matmul
trick + free-dim min-reduce.  Scalar-engine table sets force a two-pass
structure (sqrt and arctan live in different ACT table sets).

Dispatch: jit(shard_map(bass_exec)) built ONCE and cached -- the stock
run_bass_kernel_spmd path creates a fresh jax.jit closure per call and
pays a full retrace every time.
"""
import os
import sys
import numpy as np

sys.path.insert(0, '/opt/trn_rl_repo')

B, V_FULL, V_HAND, V_LOOP, N_FACES = 64, 6890, 250, 20, 500
P = V_HAND + 1          # 251 points/verts per hand (incl. lid)
PPAD = 256
NCORES = 8
NB = B // NCORES        # local batches per core
NBD = NB * 2            # (batch, dir) pairs per core
NBLK = NBD * 2          # blocks per core: x2 point-chunks of 128
SUPER = 8               # blocks per two-pass super-group
F = N_FACES
HALF_PI = float(np.pi / 2)

_compiled = None        # cached compiled program across kernel() calls
_dispatch = None
last_exec_time_ns = None


# --------------------------------------------------------------------------
# host prep: tiny uploads only
# --------------------------------------------------------------------------

def _host_prep2(inputs):
    verts = np.asarray(inputs['verts_batch'], dtype=np.float32)

    pts = {}
    for d, (hi, li) in enumerate([
            ('hand_verts_inds_left', 'hand_loop_verts_inds_left'),
            ('hand_verts_inds_right', 'hand_loop_verts_inds_right')]):
        h = verts[:, np.asarray(inputs[hi])]                    # [B,250,3]
        lid = verts[:, np.asarray(inputs[li])].mean(
            axis=1, keepdims=True, dtype=np.float32)
        pts[d] = np.concatenate([h, lid], axis=1)               # [B,251,3]

    # rows [x, y, z, |p|^2]; the -2 scale and the ones row are added on device
    gpts = np.full((NCORES, 4, NB, 2, PPAD), 1e3, np.float32)
    gpts[:, 3] = 3e6
    for d in range(2):
        pr = pts[d].reshape(NCORES, NB, P, 3).transpose(0, 3, 1, 2)
        gpts[:, 0:3, :, d, :P] = pr
        gpts[:, 3, :, d, :P] = (pr * pr).sum(axis=1)

    # faces of the OTHER hand per dir, corner-major, as floats (pad 300)
    facesf = np.full((2, 3, 512), 300.0, np.float32)
    of = {0: np.asarray(inputs['hand_faces_right']),
          1: np.asarray(inputs['hand_faces_left'])}
    for d in range(2):
        facesf[d, :, :F] = of[d].T.astype(np.float32)

    # selection master: P[k,h] = master[:, 128-k+128h : 256-k+128h]
    master = np.zeros((8, 384), np.float32)
    for nb in range(8):
        master[nb, nb * 32 + 128] = 1.0

    iota2 = (np.arange(128, dtype=np.float32)[:, None]
             + np.array([0.0, 128.0], np.float32)[None, :])     # [128,2]

    pmm = np.zeros((5, 16), np.float32)
    pmm[0, 0] = pmm[1, 1] = pmm[2, 2] = -0.5    # M5T cols 0-4
    pmm[4, 3] = 1.0
    pmm[3, 4] = 1.0
    for k in range(5):
        pmm[k, 5 + k] = 1.0                     # I5 cols 5-9

    return {
        "ptsT": gpts.reshape(NCORES * 4, NBD, PPAD),
        "faces": facesf.reshape(1, 3072),
        "master": master,
        "iota": iota2,
        "pmm": pmm,
    }


# --------------------------------------------------------------------------
# device kernel
# --------------------------------------------------------------------------

def _kernel_body(tc, ptsT_d, faces_d, master_d, iota_d, pmm_d, loss_d,
                 dbg=None):
    import concourse.mybir as mybir
    nc = tc.nc
    fp32 = mybir.dt.float32
    AF = mybir.ActivationFunctionType
    OP = mybir.AluOpType
    AX = mybir.AxisListType.X

    with tc.tile_pool(name="const", bufs=1) as cpool:
        # ---- persistent tiles --------------------------------------------
        # lhsT4: moving rows [-2x,-2y,-2z,1,|p|^2] built from raw xyz,
        # replicated at the four 32-partition offsets.
        lhsT4 = cpool.tile([128, NBD, PPAD], fp32)
        with tc.tile_pool(name="lhsTbuild", bufs=1) as lbp:
            onesrow = lbp.tile([1, NBD, PPAD], fp32, tag="onesrow")
            nc.vector.memset(onesrow[:], 1.0)
            for j in range(4):
                nc.sync.dma_start(lhsT4[32 * j:32 * j + 3], ptsT_d[0:3])
                nc.sync.dma_start(lhsT4[32 * j + 4:32 * j + 5], ptsT_d[3:4])
                nc.sync.dma_start(lhsT4[32 * j + 3:32 * j + 4], onesrow[:])
                nc.vector.tensor_scalar(lhsT4[32 * j:32 * j + 3],
                                        lhsT4[32 * j:32 * j + 3],
                                        -2.0, None, OP.mult)
        mrhs4 = cpool.tile([128, NBD, PPAD], fp32)
        # winding rhs group tiles [128, 512], partition (nb%4)*32+k
        gsb = [[[cpool.tile([128, 512], fp32, tag=f"g{d}{c}{h}", name=f"g{d}{c}{h}")
                 for h in range(2)] for c in range(3)] for d in range(2)]
        midsb = [[[cpool.tile([128, 512], fp32, tag=f"m{d}{gi}{h}", name=f"m{d}{gi}{h}")
                   for h in range(2)] for gi in range(3)] for d in range(2)]
        g6sb = [[cpool.tile([128, 512], fp32, tag=f"s{d}{h}", name=f"s{d}{h}")
                 for h in range(2)] for d in range(2)]
        ones = cpool.tile([128, 1], fp32)
        nc.vector.memset(ones[:], 1.0)
        sacc = cpool.tile([128, NBLK], fp32)     # per block: sum_f half-angle
        minda = cpool.tile([128, NBLK], fp32)    # per block: clamped min d^2

        # =============== construction phase ===============================
        with (
            tc.tile_pool(name="prep", bufs=1) as prp,
            tc.tile_pool(name="prept", bufs=2) as prt,
            tc.tile_pool(name="preps", bufs=1, space="PSUM") as pps,
        ):
            faces_sb = prp.tile([1, 3072], fp32, tag="faces")
            nc.sync.dma_start(faces_sb[:], faces_d[:])
            master_sb = prp.tile([8, 384], fp32, tag="master")
            nc.sync.dma_start(master_sb[:], master_d[:])
            iota_sb = prp.tile([128, 2], fp32, tag="iota")
            nc.sync.dma_start(iota_sb[:], iota_d[:])
            pmm_sb = prp.tile([5, 16], fp32, tag="pmm")
            nc.sync.dma_start(pmm_sb[:], pmm_d[:])
            ones1 = prp.tile([1, 128], fp32, tag="ones1")
            nc.vector.memset(ones1[:], 1.0)

            # S1: mrhs4 (other-hand vertex rows) via M5T row-mix, at the
            # four 32-partition offsets needed by the per-batch matmuls.
            for bd in range(NBD):
                nb, d = divmod(bd, 2)
                src = lhsT4[0:5, nb * 2 + (1 - d), :]
                ps = pps.tile([128, PPAD], fp32, tag="mrps")
                for j in range(4):
                    nc.tensor.matmul(ps[32 * j:32 * j + 5, :],
                                     pmm_sb[0:5, 0:5], src,
                                     tile_position=(0, 32 * j))
                for j in range(4):
                    nc.scalar.activation(mrhs4[32 * j:32 * j + 5, bd, :],
                                         ps[32 * j:32 * j + 5, :], AF.Copy)

            # S2: V5T[d][ch] [vert(128), nb*32+k] via PE transposes
            v5t = [[prp.tile([128, 256], fp32, tag=f"v{d}{ch}", name=f"v{d}{ch}")
                    for ch in range(2)] for d in range(2)]
            for d in range(2):
                for ch in range(2):
                    ps = pps.tile([128, 256], fp32, tag="v5ps")
                    for nb in range(NB):
                        bd = nb * 2 + d
                        nc.tensor.matmul(
                            ps[:, nb * 32:nb * 32 + 5],
                            mrhs4[0:5, bd, ch * 128:(ch + 1) * 128],
                            pmm_sb[0:5, 5:10])
                    nc.vector.memset(v5t[d][ch][:], 0.0)
                    for nb in range(NB):
                        nc.scalar.activation(
                            v5t[d][ch][:, nb * 32:nb * 32 + 5],
                            ps[:, nb * 32:nb * 32 + 5], AF.Copy)

            # S3-S5a under a scoped one-hot pool; coord outlives it
            with tc.tile_pool(name="coordp", bufs=1) as cop:
                coord = [[[cop.tile([8, 512], fp32, tag=f"c{d}{c}{k}",
                                    name=f"c{d}{c}{k}")
                           for k in range(3)] for c in range(3)]
                         for d in range(2)]
                with tc.tile_pool(name="ohp", bufs=1) as ohp:
                    # S3: one-hot gather matrices oh[d][c][ch]
                    oh = [[[ohp.tile([128, 512], fp32, tag=f"o{d}{c}{ch}",
                                     name=f"o{d}{c}{ch}")
                            for ch in range(2)] for c in range(3)]
                          for d in range(2)]
                    for d in range(2):
                        for c in range(3):
                            ps = pps.tile([128, 512], fp32, tag="fbps")
                            nc.tensor.matmul(
                                ps[:], ones1[0:1, :],
                                faces_sb[0:1, (d * 3 + c) * 512:
                                         (d * 3 + c + 1) * 512])
                            for ch in range(2):
                                nc.vector.tensor_scalar(
                                    oh[d][c][ch][:], ps[:],
                                    iota_sb[:, ch:ch + 1], None,
                                    OP.is_equal)

                    # S4: corner groups 0-2 (also reused for mids)
                    for d in range(2):
                        for c in range(3):
                            for h in range(2):
                                ps = pps.tile([128, 512], fp32, tag="gps")
                                nc.tensor.matmul(
                                    ps[:, :F],
                                    v5t[d][0][:, h * 128:(h + 1) * 128],
                                    oh[d][c][0][:, :F],
                                    start=True, stop=False)
                                nc.tensor.matmul(
                                    ps[:, :F],
                                    v5t[d][1][:, h * 128:(h + 1) * 128],
                                    oh[d][c][1][:, :F],
                                    start=False, stop=True)
                                nc.scalar.activation(gsb[d][c][h][:, :F],
                                                     ps[:, :F], AF.Copy)

                    # S5a: coord gathers for BOTH dirs
                    for d in range(2):
                        for c in range(3):
                            for k in range(3):
                                ps8 = pps.tile([8, 512], fp32, tag="cps")
                                nc.tensor.matmul(
                                    ps8[:, :F], v5t[d][0][:, k::32],
                                    oh[d][c][0][:, :F],
                                    start=True, stop=False)
                                nc.tensor.matmul(
                                    ps8[:, :F], v5t[d][1][:, k::32],
                                    oh[d][c][1][:, :F],
                                    start=False, stop=True)
                                nc.scalar.activation(coord[d][c][k][:, :F],
                                                     ps8[:, :F], AF.Copy)

                # S5b-S7 per dir: bilinear chain, mids, group 6
                for d in range(2):
                    with (
                        tc.tile_pool(name=f"chain{d}", bufs=1) as chp,
                        tc.tile_pool(name=f"chaint{d}", bufs=2) as cht,
                    ):
                        A, Bv, C = coord[d]
                        # bxc (kept for d0), n/2 rows accumulated in place
                        bxc = [chp.tile([8, 512], fp32, tag=f"b{k}",
                                        name=f"b{k}") for k in range(3)]
                        nh = [chp.tile([8, 512], fp32, tag=f"n{k}",
                                       name=f"n{k}") for k in range(3)]

                        def crossk(U, V, out, k):
                            k1, k2 = (k + 1) % 3, (k + 2) % 3
                            p1 = cht.tile([8, 512], fp32, tag="p1")
                            p2 = cht.tile([8, 512], fp32, tag="p2")
                            nc.vector.tensor_tensor(
                                p1[:, :F], U[k1][:, :F], V[k2][:, :F],
                                OP.mult)
                            nc.vector.tensor_tensor(
                                p2[:, :F], U[k2][:, :F], V[k1][:, :F],
                                OP.mult)
                            nc.vector.tensor_tensor(
                                out[:, :F], p1[:, :F], p2[:, :F],
                                OP.subtract)

                        for k in range(3):
                            crossk(Bv, C, bxc[k], k)
                            t = cht.tile([8, 512], fp32, tag="nt")
                            u = cht.tile([8, 512], fp32, tag="nu")
                            crossk(A, Bv, t, k)             # axb_k
                            crossk(C, A, u, k)              # cxa_k
                            nc.vector.tensor_tensor(t[:, :F], t[:, :F],
                                                    bxc[k][:, :F], OP.add)
                            nc.vector.tensor_tensor(t[:, :F], t[:, :F],
                                                    u[:, :F], OP.add)
                            nc.vector.tensor_scalar(nh[k][:, :F], t[:, :F],
                                                    0.5, None, OP.mult)
                        # d0 = A.(BxC)
                        d0 = chp.tile([8, 512], fp32, tag="d0")
                        t = cht.tile([8, 512], fp32, tag="nt")
                        nc.vector.tensor_tensor(d0[:, :F], A[0][:, :F],
                                                bxc[0][:, :F], OP.mult)
                        nc.vector.tensor_tensor(t[:, :F], A[1][:, :F],
                                                bxc[1][:, :F], OP.mult)
                        nc.vector.tensor_tensor(d0[:, :F], d0[:, :F],
                                                t[:, :F], OP.add)
                        nc.vector.tensor_tensor(t[:, :F], A[2][:, :F],
                                                bxc[2][:, :F], OP.mult)
                        nc.vector.tensor_tensor(d0[:, :F], d0[:, :F],
                                                t[:, :F], OP.add)
                        # em[pair] = -0.5*|U-V|^2
                        pairs = [(A, Bv), (Bv, C), (C, A)]
                        em = [chp.tile([8, 512], fp32, tag=f"e{gi}",
                                       name=f"e{gi}") for gi in range(3)]
                        for gi, (U, V) in enumerate(pairs):
                            acc = cht.tile([8, 512], fp32, tag="p1")
                            for k in range(3):
                                s = cht.tile([8, 512], fp32, tag="p2")
                                nc.vector.tensor_tensor(
                                    s[:, :F], U[k][:, :F], V[k][:, :F],
                                    OP.subtract)
                                if k == 0:
                                    nc.vector.tensor_tensor(
                                        acc[:, :F], s[:, :F], s[:, :F],
                                        OP.mult)
                                else:
                                    sq = cht.tile([8, 512], fp32, tag="nu")
                                    nc.vector.tensor_tensor(
                                        sq[:, :F], s[:, :F], s[:, :F],
                                        OP.mult)
                                    nc.vector.tensor_tensor(
                                        acc[:, :F], acc[:, :F], sq[:, :F],
                                        OP.add)
                            nc.vector.tensor_scalar(em[gi][:, :F],
                                                    acc[:, :F],
                                                    -0.5, None, OP.mult)

                        # S6: mid groups 3-5 = 0.5*(gA+gB) + scatter(em)
                        cpairs = [(0, 1), (1, 2), (2, 0)]
                        for gi, (ca, cb) in enumerate(cpairs):
                            for h in range(2):
                                ps = pps.tile([128, 512], fp32,
                                              tag="corrps")
                                off = 125 + 128 * h   # k=3 slice of master
                                nc.tensor.matmul(
                                    ps[:, :F],
                                    master_sb[:, off:off + 128],
                                    em[gi][:, :F])
                                t1 = cht.tile([128, 512], fp32, tag="midt")
                                nc.vector.tensor_tensor(
                                    t1[:, :F], gsb[d][ca][h][:, :F],
                                    gsb[d][cb][h][:, :F], OP.add)
                                nc.vector.scalar_tensor_tensor(
                                    midsb[d][gi][h][:, :F], t1[:, :F], 0.5,
                                    ps[:, :F], OP.mult, OP.add)

                        # S7: group 6 = scatter(n/2 rows) + scatter(d0)
                        for h in range(2):
                            ps = pps.tile([128, 512], fp32, tag="g6ps")
                            for k in range(3):
                                off = 128 - k + 128 * h
                                nc.tensor.matmul(
                                    ps[:, :F],
                                    master_sb[:, off:off + 128],
                                    nh[k][:, :F],
                                    start=(k == 0), stop=False)
                            off = 125 + 128 * h
                            nc.tensor.matmul(ps[:, :F],
                                             master_sb[:, off:off + 128],
                                             d0[:, :F], start=False,
                                             stop=True)
                            nc.scalar.activation(g6sb[d][h][:, :F],
                                                 ps[:, :F], AF.Copy)

        if dbg is not None:
            for d in range(2):
                for c in range(3):
                    for h in range(2):
                        nc.sync.dma_start(dbg[f"g{d}{c}{h}"], gsb[d][c][h][:, :F])
                for gi in range(3):
                    for h in range(2):
                        nc.sync.dma_start(dbg[f"m{d}{gi}{h}"],
                                          midsb[d][gi][h][:, :F])
                for h in range(2):
                    nc.sync.dma_start(dbg[f"s{d}{h}"], g6sb[d][h][:, :F])
            nc.sync.dma_start(dbg["mrhs4"], mrhs4[0:5])
        dbg_post = dbg

        # =============== main winding / min-dist loop =====================
        with (
            tc.tile_pool(name="store", bufs=1) as spool,
            tc.tile_pool(name="iface", bufs=2) as ipool,
            tc.tile_pool(name="dve", bufs=1) as vpool,
        ):
            denoms = spool.tile([128, SUPER, 512], fp32)
            tts = spool.tile([128, SUPER, 512], fp32)

            def pass_a(ppool, i, j):
                bd, ch = divmod(i, 2)
                nb, d = divmod(bd, 2)
                h, q = divmod(nb, 4)
                lhs = lhsT4[q * 32:q * 32 + 5, bd, ch * 128:(ch + 1) * 128]
                G = [gsb[d][0][h], gsb[d][1][h], gsb[d][2][h],
                     midsb[d][0][h], midsb[d][1][h], midsb[d][2][h],
                     g6sb[d][h]]

                wind = ppool.tile([128, 7, 512], fp32, tag="wind")
                md = ppool.tile([128, 256], fp32, tag="md")

                for g in range(7):
                    nc.tensor.matmul(wind[:, g, :F], lhs,
                                     G[g][q * 32:q * 32 + 5, :F],
                                     tile_position=(q * 32, 0))
                nc.tensor.matmul(md[:, :P], lhs,
                                 mrhs4[q * 32:q * 32 + 5, bd, :P],
                                 tile_position=(q * 32, 0))

                # min-distance: free-dim min, clamp at 0 (matmul roundoff)
                mind = vpool.tile([128, 1], fp32, tag="mind")
                nc.vector.tensor_reduce(mind[:], md[:, :P], AX, OP.min)
                nc.vector.tensor_scalar(minda[:, i:i + 1], mind[:], 0.0,
                                        None, OP.max)

                # norms: clamp squared lengths at 0, sqrt
                rl = ipool.tile([128, 3, 512], fp32, tag="rl")
                for g in range(3):
                    nc.scalar.activation(rl[:, g, :F], wind[:, g, :F],
                                         AF.Relu)
                la = ipool.tile([128, 512], fp32, tag="la")
                lb = ipool.tile([128, 512], fp32, tag="lb")
                lc = ipool.tile([128, 512], fp32, tag="lc")
                nc.scalar.activation(la[:, :F], rl[:, 0, :F], AF.Sqrt)
                nc.scalar.activation(lb[:, :F], rl[:, 1, :F], AF.Sqrt)
                nc.scalar.activation(lc[:, :F], rl[:, 2, :F], AF.Sqrt)
                dets = ipool.tile([128, 512], fp32, tag="dets")
                nc.scalar.activation(dets[:, :F], wind[:, 6, :F], AF.Copy)

                # denominator chain (DVE); PSUM reads scheduled early
                u = vpool.tile([128, 512], fp32, tag="u")
                r4 = vpool.tile([128, 512], fp32, tag="r4")
                s5 = vpool.tile([128, 512], fp32, tag="s5")
                v = vpool.tile([128, 512], fp32, tag="v")
                w = vpool.tile([128, 512], fp32, tag="w")
                t6 = vpool.tile([128, 512], fp32, tag="t6")
                nc.vector.tensor_tensor(r4[:, :F], wind[:, 4, :F],
                                        la[:, :F], OP.mult)
                nc.vector.tensor_tensor(s5[:, :F], wind[:, 5, :F],
                                        lb[:, :F], OP.mult)
                nc.vector.tensor_tensor(u[:, :F], la[:, :F], lb[:, :F],
                                        OP.mult)
                nc.vector.tensor_tensor(v[:, :F], u[:, :F], wind[:, 3, :F],
                                        OP.add)

                # rest of the chain is SBUF-only
                w_ = w[:, :F]
                nc.vector.tensor_tensor(w_, v[:, :F], lc[:, :F], OP.mult)
                nc.vector.tensor_tensor(t6[:, :F], r4[:, :F], s5[:, :F],
                                        OP.add)
                den = denoms[:, j, :F]
                nc.vector.tensor_tensor(den, w_, t6[:, :F], OP.add)

                # half-angle atan2 range reduction: tt = det / (rho + |den|)
                xx = ipool.tile([128, 512], fp32, tag="xx")
                yy = ipool.tile([128, 512], fp32, tag="yy")
                ss = vpool.tile([128, 512], fp32, tag="ss", bufs=2)
                rho = ipool.tile([128, 512], fp32, tag="rho")
                axd = ipool.tile([128, 512], fp32, tag="axd")
                dd = vpool.tile([128, 512], fp32, tag="dd")
                rd = vpool.tile([128, 512], fp32, tag="rd")
                nc.scalar.activation(xx[:, :F], den, AF.Square)
                nc.scalar.activation(yy[:, :F], dets[:, :F], AF.Square)
                nc.vector.scalar_tensor_tensor(ss[:, :F], xx[:, :F], 1e-20,
                                               yy[:, :F], OP.add, OP.add)
                nc.scalar.activation(rho[:, :F], ss[:, :F], AF.Sqrt)
                nc.scalar.activation(axd[:, :F], den, AF.Abs)
                nc.vector.tensor_tensor(dd[:, :F], rho[:, :F], axd[:, :F],
                                        OP.add)
                nc.vector.reciprocal_approx_fast(rd[:, :F], dd[:, :F])
                nc.vector.tensor_tensor(tts[:, j, :F], dets[:, :F],
                                        rd[:, :F], OP.mult)

            def pass_b(i, j):
                den = denoms[:, j, :F]
                tt = tts[:, j, :F]
                sgn = ipool.tile([128, 512], fp32, tag="sgn")
                spi = ipool.tile([128, 512], fp32, tag="spi")
                atn = ipool.tile([128, 512], fp32, tag="atn")
                c0 = vpool.tile([128, 512], fp32, tag="c0")
                c1 = vpool.tile([128, 512], fp32, tag="c1")
                sd = vpool.tile([128, 512], fp32, tag="sd")
                nc.scalar.activation(sgn[:, :F], tt, AF.Sign)
                nc.scalar.mul(spi[:, :F], sgn[:, :F], HALF_PI)
                nc.scalar.activation(atn[:, :F], tt, AF.Arctan)
                # half = atn + [den<0]*(pi/2*sign(det) - 2*atn)
                nc.vector.scalar_tensor_tensor(c0[:, :F], atn[:, :F], -2.0,
                                               spi[:, :F], OP.mult, OP.add)
                nc.vector.scalar_tensor_tensor(c1[:, :F], den, 0.0,
                                               c0[:, :F], OP.is_lt, OP.mult)
                nc.vector.scalar_tensor_tensor(sd[:, :F], atn[:, :F], 0.0,
                                               c1[:, :F], OP.add, OP.add,
                                               accum_out=sacc[:, i:i + 1])

            with tc.tile_pool(name="psum", bufs=1, space="PSUM") as ppool:
                for s in range(NBLK // SUPER):
                    for j in range(SUPER):
                        pass_a(ppool, s * SUPER + j, j)
                    tc.no_sync_barrier()
                    for j in range(SUPER):
                        pass_b(s * SUPER + j, j)
                    tc.no_sync_barrier()

            if dbg is not None:
                nc.sync.dma_start(dbg["sacc"], sacc[:])
                nc.sync.dma_start(dbg["minda"], minda[:])

            # ---------------- final: depth * inside, partition-reduce -----
            inside = cpool.tile([128, NBLK], fp32)
            depth = cpool.tile([128, NBLK], fp32)
            contrib = cpool.tile([128, NBLK], fp32)
            beps = cpool.tile([128, 1], fp32)
            nc.vector.memset(beps[:], 1e-12)
            nc.vector.tensor_scalar(inside[:], sacc[:], HALF_PI, None,
                                    OP.is_gt)
            nc.scalar.activation(depth[:], minda[:], AF.Sqrt, bias=beps[:])
            nc.vector.tensor_tensor(contrib[:], depth[:], inside[:],
                                    OP.mult)

            with tc.tile_pool(name="psum2", bufs=1, space="PSUM") as p2:
                lpsum = p2.tile([NBLK, 1], fp32)
                nc.tensor.matmul(lpsum[:], contrib[:], ones[:])
                loss_sb = cpool.tile([NBLK, 1], fp32)
                nc.scalar.activation(loss_sb[:], lpsum[:], AF.Copy)
                nc.sync.dma_start(loss_d[:], loss_sb[:])


def _build():
    global _compiled
    if _compiled is not None:
        return _compiled
    import concourse.bacc as bacc
    import concourse.mybir as mybir
    import concourse.tile as tile

    nc = bacc.Bacc("TRN2", target_bir_lowering=False, debug=False,
                   num_devices=NCORES)
    fp32 = mybir.dt.float32
    ptsT_d = nc.dram_tensor("ptsT", (4, NBD, PPAD), fp32,
                            kind="ExternalInput").ap()
    faces_d = nc.dram_tensor("faces", (1, 3072), fp32,
                             kind="ExternalInput").ap()
    master_d = nc.dram_tensor("master", (8, 384), fp32,
                              kind="ExternalInput").ap()
    iota_d = nc.dram_tensor("iota", (128, 2), fp32,
                            kind="ExternalInput").ap()
    pmm_d = nc.dram_tensor("pmm", (5, 16), fp32, kind="ExternalInput").ap()
    loss_d = nc.dram_tensor("loss", (NBLK, 1), fp32,
                            kind="ExternalOutput").ap()

    with tile.TileContext(nc) as tc:
        _kernel_body(tc, ptsT_d, faces_d, master_d, iota_d, pmm_d, loss_d)
    nc.compile()
    _compiled = nc
    return nc


# --------------------------------------------------------------------------
# dispatch: jit(shard_map(bass_exec)) built ONCE and cached.
# --------------------------------------------------------------------------

def _get_dispatch():
    global _dispatch
    if _dispatch is not None:
        return _dispatch
    import jax
    from jax.experimental.shard_map import shard_map
    from jax.sharding import Mesh, PartitionSpec
    from concourse import bass2jax
    import concourse.mybir as mybir

    nc = _build()
    bass2jax.install_neuronx_cc_hook()
    assert nc.dbg_addr is None
    part_name = nc.partition_id_tensor.name if nc.partition_id_tensor else None

    in_names, out_names, out_avals = [], [], []
    for alloc in nc.m.functions[0].allocations:
        if not isinstance(alloc, mybir.MemoryLocationSet):
            continue
        name = alloc.memorylocations[0].name
        if alloc.kind == "ExternalInput":
            if name != part_name:
                in_names.append(name)
        elif alloc.kind == "ExternalOutput":
            out_names.append(name)
            out_avals.append(jax.core.ShapedArray(
                tuple(alloc.tensor_shape), mybir.dt.np(alloc.dtype)))
    n_params = len(in_names)
    all_names = tuple(in_names) + tuple(out_names)
    if part_name is not None:
        all_names = all_names + (part_name,)

    def _body(*args):
        operands = list(args)
        if part_name is not None:
            operands.append(bass2jax.partition_id_tensor())
        outs = bass2jax._bass_exec_p.bind(
            *operands,
            out_avals=tuple(out_avals),
            in_names=all_names,
            out_names=tuple(out_names),
            lowering_input_output_aliases=(),
            sim_require_finite=True,
            sim_require_nnan=True,
            nc=nc,
        )
        return tuple(outs)

    devices = jax.devices()[:NCORES]
    mesh = Mesh(np.asarray(devices), ("core",))
    repl = {"faces", "master", "iota", "pmm"}
    in_specs = tuple(
        PartitionSpec() if n in repl else PartitionSpec("core")
        for n in in_names) + (PartitionSpec("core"),) * len(out_names)
    out_specs = (PartitionSpec("core"),) * len(out_names)
    donate = tuple(range(n_params, n_params + len(out_names)))
    sharded = jax.jit(
        shard_map(_body, mesh=mesh, in_specs=in_specs,
                  out_specs=out_specs, check_rep=False),
        donate_argnums=donate, keep_unused=True)
    _dispatch = (sharded, tuple(in_names))
    return _dispatch


# --------------------------------------------------------------------------
# entry point
# --------------------------------------------------------------------------

def kernel(**inputs) -> np.ndarray:
    global last_exec_time_ns
    feed = _host_prep2(inputs)

    if bool(int(os.environ.get("HAND_KERNEL_TRACE", "0"))):
        # profiling path: stock spmd runner with NTFF tracing
        from concourse.bass_utils import run_bass_kernel_spmd
        nc = _build()
        maps = []
        for c in range(NCORES):
            m = {"ptsT": feed["ptsT"].reshape(
                (NCORES, 4) + feed["ptsT"].shape[1:])[c].copy()}
            for k in ("faces", "master", "iota", "pmm"):
                m[k] = feed[k].copy()
            maps.append(m)
        res = run_bass_kernel_spmd(nc, maps, list(range(NCORES)), trace=True)
        last_exec_time_ns = res.exec_time_ns
        loss = np.zeros(B, np.float32)
        for c in range(NCORES):
            out = np.asarray(res.results[c]["loss"], np.float32).reshape(NBLK)
            loss[c * NB:(c + 1) * NB] = out.reshape(NB, 4).sum(axis=1)
        return loss

    sharded, in_names = _get_dispatch()
    (out,) = sharded(*[feed[n] for n in in_names],
                     np.zeros((NCORES * NBLK, 1), np.float32))
    last_exec_time_ns = None
    # block i = ((b_loc*2 + dir)*2 + chunk)
    return np.asarray(out, np.float32).reshape(B, 4).sum(axis=1)
